# revision 1
# baseline (speedup 1.0000x reference)
"""Trainium2 Bass kernel for nn_CustomLoss (2-Wasserstein-style Gaussian loss).

loss = mean((mu_p-mu_t)^2) + tr(Cp) + tr(Ct) + 2*tr(sqrtm(S2 @ Ct @ S2)),
       S2 = sqrtm(Cp),  d = 2048, packed inputs (4, 2100224), row 0 used.

Device algorithm: two scaled coupled Newton-Schulz sqrt chains in fp32r
(TensorEngine full rate), 8-way row-sharded across the NeuronCores with
AllGather (full operands for streaming) + AllToAll (column-slice delivery for
the stationary operand, avoiding core-dependent addressing in the SPMD
program). Scalar normalizers and the per-iteration scaling schedule are
host-side; the schedule is input-independent so one NEFF serves all inputs.
"""
import numpy as np

import concourse.bass as bass
import concourse.mybir as mybir
import concourse.tile as tile
from concourse.bass_utils import run_bass_kernel_spmd
from concourse.masks import make_identity

# Disable the walrus-embedded BIR simulator: ~4x faster NEFF compiles.
import concourse.bass_utils as _bu
if not getattr(_bu, "_nobirsim_patched", False):
    _orig_bvo = _bu.bir_verify_and_optimise

    def _bvo_fast(tmpdir, inp="bir.json", outp="file.neff", arch=None, *, dve_root=None):
        orig_run = _bu.run_command

        def patched_run(argv, **kw):
            argv = [a.replace("--enable-birsim=true", "--enable-birsim=false")
                    if isinstance(a, str) else a for a in argv]
            return orig_run(argv, **kw)

        _bu.run_command = patched_run
        try:
            return _orig_bvo(tmpdir, inp, outp, arch, dve_root=dve_root)
        finally:
            _bu.run_command = orig_run

    _bu.bir_verify_and_optimise = _bvo_fast
    _bu._nobirsim_patched = True

# ----------------------------------------------------------------------------
# config
D = 2048
NC = 8
SH = D // NC          # 256 rows per core
P = 128
KT = D // P           # 16 k-tiles
MB = SH // P          # 2 m-blocks per shard
NB = D // 512         # 4 n-blocks
CH = 2                # k-tiles per stream chunk
_TAG_BUFS = {"ostag": 2, "tstag": 2, "zstag": 1, "rstream": 2, "lhsT": 3}
EPS = 1e-4            # ridge (normalized units)
QCAP = 2.5            # max scaled eigenvalue (stability margin)
K1 = 10               # NS1 iterations (incl. cheap iter 1) + half-step
K2 = 12               # NS2 iterations (incl. cheap iter 1) + trace correction
F32 = mybir.dt.float32
F32R = mybir.dt.float32r
AF = mybir.ActivationFunctionType
ALU = mybir.AluOpType

_BUILD_CACHE = {}


# ----------------------------------------------------------------------------
# host: schedule
def _f(q):
    return q * (3.0 - q) ** 2 / 4.0


def _balance_s(a, b, qcap):
    """s with f(s*a) = f(s*b), s*b <= qcap, via bisection."""
    s_hi = min(qcap, 2.9999) / b
    g = lambda s: _f(s * a) - _f(s * b)
    if g(s_hi) <= 0:
        return s_hi
    lo, hi = 1e-12, s_hi
    for _ in range(80):
        mid = 0.5 * (lo + hi)
        if g(mid) > 0:
            hi = mid
        else:
            lo = mid
    return 0.5 * (lo + hi)


def make_schedule(delta, b0, iters, qcap=QCAP):
    a, b = delta, b0
    out = []
    for _ in range(iters):
        s = 1.0 if a > 0.99 * b else _balance_s(a, b, qcap)
        mu = np.sqrt(s)
        out.append((1.5 * mu, -0.5 * mu ** 3))   # (alpha, beta): T = a*I + b*P
        qa, qb = s * a, s * b
        vals = [_f(qa), _f(qb)]
        b = 1.0 if qa <= 1.0 <= qb else max(vals)
        a = min(vals)
    return out


# ----------------------------------------------------------------------------
# host: input prep
def _unpack_row(v):
    mu = v[:D].astype(np.float64)
    tri = v[D:]
    C = np.zeros((D, D), np.float32)
    iu, ju = np.triu_indices(D)
    C[iu, ju] = tri
    C[ju, iu] = tri
    return mu, C


def _power_iter_sym(C, iters=60):
    rng = np.random.default_rng(12345)
    x = rng.standard_normal(D)
    C64 = C.astype(np.float64)
    lam = 1.0
    for _ in range(iters):
        y = C64 @ x
        lam = np.linalg.norm(y)
        x = y / lam
    return float(lam)


def _power_iter_prod(Cp, Ct, iters=60):
    rng = np.random.default_rng(54321)
    x = rng.standard_normal(D)
    Cp64 = Cp.astype(np.float64)
    Ct64 = Ct.astype(np.float64)
    lam = 1.0
    for _ in range(iters):
        y = Cp64 @ (Ct64 @ x)
        lam = np.linalg.norm(y)
        x = y / lam
    return float(lam)


# ----------------------------------------------------------------------------
# walrus workaround: this build allows only ONE sync-wait per instruction
class PatchedTileContext(tile.TileContext):
    def _drain_and_barrier(self, tick_clock, wait_clock):
        from concourse.vector_clock import ScopedClock

        probe = self.nc.sync.nop(nofuse=True)
        wait_clock.add_sem_waits(
            probe.ins, ScopedClock({None: tick_clock.global_clock})
        )
        si = probe.ins.sync_info
        waits = list(si.on_wait) if si is not None else []
        if len(waits) > 1:
            si.on_wait = [waits[0]]
            for w in waits[1:]:
                n2 = self.nc.sync.nop(nofuse=True)
                si2 = n2.ins.sync_info
                if si2 is None:
                    n2.ins.sync_info = mybir.SyncInfo(on_wait=[w], on_update=[])
                else:
                    si2.on_wait = [w]
        self.nc.sync.drain()
        self.nc.all_engine_barrier()
        assert self.sems is not None
        popped = self.nc._tile_sem_poison_stack.pop()
        assert popped is self._sem_poison
        self.nc.clear_and_free_semaphores(list(self.sems.allocated().values()))
        self.nc.all_engine_barrier()


def legalize_single_wait(nc):
    uid = 0
    for fn in nc.m.functions:
        for blk in fn.blocks:
            il = blk.instructions
            if not any(
                i.sync_info is not None and len(i.sync_info.on_wait) > 1 for i in il
            ):
                continue
            new = []
            for ins in il:
                si = ins.sync_info
                waits = list(si.on_wait) if si is not None else []
                if len(waits) > 1:
                    si.on_wait = [waits[-1]]
                    for w in waits[:-1]:
                        nop = mybir.InstNoOp(
                            name=f"legalize-wait-{uid}",
                            engine=ins.engine,
                            sync_info=mybir.SyncInfo(on_wait=[w], on_update=[]),
                        )
                        uid += 1
                        new.append(nop)
                new.append(ins)
            blk.instructions = new


# ----------------------------------------------------------------------------
# device program builder
class _B:
    """Builder state."""

    def __init__(self, nc, tc, dram, sb, psum):
        self.nc, self.tc = nc, tc
        self.dram, self.sb, self.psum = dram, sb, psum
        self.uid = 0
        self.ident = None    # [P, P] identity f32
        self.epsrow = None   # [P, MB, D] eps*I row slab (per-core input)

    def u(self, s):
        self.uid += 1
        return f"{s}_{self.uid}"


def _stream_view(full_ap):
    """[D, D] dram AP -> [P, NCH, CH, D] chunked k-tile stream view."""
    return full_ap.rearrange("(ch kb p) n -> p ch kb n", p=P, kb=CH)


def _lhsT_view(a2a_ap):
    """[D, SH] dram AP (A2A out, flat) -> [P, KT, SH]."""
    return a2a_ap.rearrange("(k p) m -> p k m", p=P)


def _mm_shard(b: _B, lhsT_sb, rhs_chunks, scale, eps_coef, tag="ostag"):
    """out_stag[P, MB, D] = (lhsT^T @ rhs) * scale (+ eps_coef * epsrow).

    lhsT_sb: [P, KT, SH] f32 sbuf; rhs_chunks: [P, NCHUNK, CH, D] dram view.
    scale: float or AP. eps_coef: None or float g (adds g * epsrow).
    """
    nc = b.nc
    stag = b.sb.tile([P, MB, D], F32R, tag=tag, name=b.u(tag), bufs=_TAG_BUFS[tag])
    ps = [
        b.psum.tile([P, 512], F32, tag="mmps", name=b.u("ps"))
        for _ in range(MB * NB)
    ]
    for ch in range(KT // CH):
        rt = b.sb.tile([P, CH, D], F32R, tag="rstream", name=b.u("rt"), bufs=_TAG_BUFS["rstream"])
        nc.sync.dma_start(out=rt[:], in_=rhs_chunks[:, ch])
        for kk in range(CH):
            k = ch * CH + kk
            for m in range(MB):
                for n in range(NB):
                    nc.tensor.matmul(
                        ps[m * NB + n][:],
                        lhsT_sb[:, k, m * P:(m + 1) * P],
                        rt[:, kk, n * 512:(n + 1) * 512],
                        start=(k == 0),
                        stop=(k == KT - 1),
                    )
    for m in range(MB):
        for n in range(NB):
            if eps_coef is not None:
                # add (eps_coef/scale) * epsrow into psum pre-eviction so the
                # scaled eviction yields  scale*psum + eps_coef*epsrow
                nc.vector.scalar_tensor_tensor(
                    ps[m * NB + n][:],
                    b.epsrow[:, m, n * 512:(n + 1) * 512],
                    float(eps_coef) / _scale_const(scale),
                    ps[m * NB + n][:],
                    ALU.mult,
                    ALU.add,
                )
            nc.scalar.activation(
                stag[:, m, n * 512:(n + 1) * 512],
                ps[m * NB + n][:],
                AF.Copy,
                scale=scale,
            )
    return stag


def _scale_const(scale):
    assert isinstance(scale, (int, float)), "eps_coef requires constant scale"
    return float(scale)


def _transpose_shard(b: _B, stag):
    """[P, MB, D] staging (rows shard of X) -> [P, KT, SH] = X^T[:, shard cols]."""
    nc = b.nc
    tt = b.sb.tile([P, KT, SH], F32R, tag="lhsT", name=b.u("tt"), bufs=_TAG_BUFS["lhsT"])
    for k in range(KT):
        for m in range(MB):
            tp = b.psum.tile([P, 512], F32R, tag="mmps", name=b.u("tps"))
            nc.tensor.transpose(
                tp[:, 0:P], stag[:, m, k * P:(k + 1) * P], b.ident[:]
            )
            nc.scalar.copy(tt[:, k, m * P:(m + 1) * P], tp[:, 0:P])
    return tt


def _load_lhsT(b: _B, dram_flat_ap):
    """DMA [D, SH] dram -> [P, KT, SH] sbuf."""
    t = b.sb.tile([P, KT, SH], F32R, tag="lhsT", name=b.u("lh"), bufs=_TAG_BUFS["lhsT"])
    b.nc.sync.dma_start(out=t[:], in_=_lhsT_view(dram_flat_ap))
    return t


def _bounce_and_gather(b: _B, stag, want_a2a, name):
    """Write staging to DRAM, AllGather full (+ optionally AllToAll col-slice).

    Returns (full_dram_ap [D, D], a2a_out_ap [D, SH] or None).
    """
    nc = b.nc
    bounce = b.dram.tile([SH, D], F32R, name=b.u(f"bn_{name}"), tag="d_bn", bufs=4)
    nc.gpsimd.dma_start(
        out=bounce[:].rearrange("(m p) n -> p m n", p=P), in_=stag[:]
    )
    full = b.dram.tile([D, D], F32R, name=b.u(f"fl_{name}"), addr_space="Shared", tag="d_fl", bufs=4)
    nc.gpsimd.collective_compute(
        "AllGather",
        ALU.bypass,
        replica_groups=[list(range(NC))],
        ins=[bounce[:]],
        outs=[full[:]],
    )
    a2a_out = None
    if want_a2a:
        a2a_in = b.dram.tile([NC, SH, SH], F32R, name=b.u(f"ai_{name}"), tag="d_ai", bufs=4)
        for j in range(NC):
            nc.gpsimd.dma_start(
                out=a2a_in[j].rearrange("(m p) n -> p m n", p=P),
                in_=stag[:, :, j * SH:(j + 1) * SH],
            )
        a2a_out = b.dram.tile([NC * SH, SH], F32R, name=b.u(f"ao_{name}"), tag="d_ao", bufs=4)
        nc.gpsimd.collective_compute(
            "AllToAll",
            ALU.bypass,
            replica_groups=[list(range(NC))],
            ins=[a2a_in[:]],
            outs=[a2a_out[:]],
        )
    return full[:], (a2a_out[:] if a2a_out is not None else None)


def _ns_chain(b: _B, a_col_lhsT_sb, a_row_stag, sched, name):
    """Run a scaled NS chain. Inputs:
      a_col_lhsT_sb: [P, KT, SH] sbuf = A[:, shard cols]  (lhsT of A)
      a_row_stag:    [P, MB, D] sbuf = A[shard rows, :]   (row slab of A)
    Returns dict with Yfull, Zfull (dram APs), Y_a2a, Z_a2a, Y_stag (sbuf).
    """
    nc = b.nc
    al0, be0 = sched[0]
    # iter 1: T0 = al0*I + be0*A (sharded, elementwise); Z1 = T0; Y1 = A @ T0
    t0f = b.sb.tile([P, MB, D], F32, tag="f32tmp", name=b.u("t0f"), bufs=1)
    t0 = b.sb.tile([P, MB, D], F32R, tag="ostag", name=b.u("t0"), bufs=_TAG_BUFS["ostag"])
    for m in range(MB):
        nc.scalar.mul(t0f[:, m, :], a_row_stag[:, m, :].bitcast(F32), float(be0))
        nc.vector.scalar_tensor_tensor(
            t0f[:, m, :], b.epsrow[:, m, :], float(al0 / EPS),
            t0f[:, m, :], ALU.mult, ALU.add,
        )
        nc.scalar.copy(t0[:, m, :], t0f[:, m, :])
    t0_full, t0_a2a = _bounce_and_gather(b, t0, True, f"{name}t0")
    y_stag = _mm_shard(b, a_col_lhsT_sb, _stream_view(t0_full), 1.0, None)
    y_full, y_a2a = _bounce_and_gather(b, y_stag, True, f"{name}y1")
    st = dict(Yfull=y_full, Y_a2a=y_a2a, Zfull=t0_full, Z_a2a=t0_a2a, Y_stag=y_stag)

    for it in range(1, len(sched)):
        al, be = sched[it]
        lh_z = _get_lhsT(b, st, "Z")
        lh_y = _get_lhsT(b, st, "Y")
        # P = Z @ Y ; T = al*I + be*P  (keep T staging for local transpose)
        t_stag = _mm_shard(b, lh_z, _get_stream(b, st, "Y"), float(be), al / EPS,
                           tag="tstag")
        t_full, _ = _bounce_and_gather(b, t_stag, False, f"{name}t{it}")
        # Z' = T @ Z : lhsT = T^T[:, shard] = transpose of own T staging
        lh_tt = _transpose_shard(b, t_stag)
        z_stag = _mm_shard(b, lh_tt, _get_stream(b, st, "Z"), 1.0, None,
                           tag="zstag")
        # Y' = Y @ T
        y_stag = _mm_shard(b, lh_y, _stream_view(t_full), 1.0, None)
        # batched gather of (Y', Z')
        bounce = b.dram.tile([2 * SH, D], F32R, name=b.u("bnyz"), tag="d_bnyz", bufs=4)
        nc.gpsimd.dma_start(
            out=bounce[:].rearrange("(t m p) n -> t p m n", t=2, p=P)[0],
            in_=y_stag[:])
        nc.gpsimd.dma_start(
            out=bounce[:].rearrange("(t m p) n -> t p m n", t=2, p=P)[1],
            in_=z_stag[:])
        full = b.dram.tile([NC * 2 * SH, D], F32R, name=b.u("flyz"),
                           addr_space="Shared", tag="d_flyz", bufs=4)
        nc.gpsimd.collective_compute(
            "AllGather", ALU.bypass, replica_groups=[list(range(NC))],
            ins=[bounce[:]], outs=[full[:]],
        )
        a2a_in = b.dram.tile([NC, 2, SH, SH], F32R, name=b.u("aiyz"), tag="d_aiyz", bufs=4)
        for j in range(NC):
            nc.gpsimd.dma_start(
                out=a2a_in[j, 0].rearrange("(m p) n -> p m n", p=P),
                in_=y_stag[:, :, j * SH:(j + 1) * SH])
            nc.gpsimd.dma_start(
                out=a2a_in[j, 1].rearrange("(m p) n -> p m n", p=P),
                in_=z_stag[:, :, j * SH:(j + 1) * SH])
        a2a_out = b.dram.tile([NC, 2, SH, SH], F32R, name=b.u("aoyz"), tag="d_aoyz", bufs=4)
        nc.gpsimd.collective_compute(
            "AllToAll", ALU.bypass, replica_groups=[list(range(NC))],
            ins=[a2a_in[:]], outs=[a2a_out[:]],
        )
        # views: full rows = (c, t, m p); Y = t 0, Z = t 1
        fv = full[:].rearrange("(c t kb p) n -> t p c kb n", t=2, kb=CH, p=P)
        av = a2a_out[:].rearrange("s t (kb p) m -> t p s kb m", kb=CH, p=P)
        st = dict(
            Yfull=fv[0], Zfull=fv[1],           # [P, NC, CH, D] chunk views
            Y_a2a=av[0], Z_a2a=av[1],           # [P, s, kb, SH] 4d lhsT views
            Y_stag=y_stag, Z_stag=z_stag,
            chunked=True,
        )
    return st


def _load_lhsT4(b: _B, view4):
    """DMA [P, s, kb, SH] 4d view -> [P, KT, SH] sbuf (k = s*CH + kb)."""
    t = b.sb.tile([P, KT, SH], F32R, tag="lhsT", name=b.u("lh4"), bufs=_TAG_BUFS["lhsT"])
    for s in range(NC):
        b.nc.sync.dma_start(
            out=t[:, s * CH:(s + 1) * CH, :], in_=view4[:, s]
        )
    return t


def _get_lhsT(b, st, key):
    v = st[f"{key}_a2a"]
    if st.get("chunked"):
        return _load_lhsT4(b, v)
    return _load_lhsT(b, v)


def _get_stream(b, st, key):
    v = st[f"{key}full"]
    if st.get("chunked"):
        return v
    return _stream_view(v)


def build_device_program(k1, k2, repeat=1):
    sched1 = make_schedule(EPS, 1.0 + EPS, k1)
    sched2 = make_schedule(EPS, 1.0 + EPS, k2)

    nc = bass.Bass(num_devices=NC)
    with PatchedTileContext(nc) as tc:
        with tc.tile_pool(name="dram", bufs=1, space="DRAM") as dram, \
             tc.tile_pool(name="sb", bufs=1) as sb_const, \
             tc.tile_pool(name="sbw", bufs=3) as sbw, \
             tc.tile_pool(name="psum", bufs=8, space="PSUM") as psum:

            b = _B(nc, tc, dram, sbw, psum)

            # --- inputs
            a1col = dram.tile([D, SH], F32R, kind="ExternalInput", name="a1col", uniquify=False)
            a1row = dram.tile([SH, D], F32, kind="ExternalInput", name="a1row", uniquify=False)
            ctcol = dram.tile([D, SH], F32R, kind="ExternalInput", name="ctcol", uniquify=False)
            epsrow_d = dram.tile([SH, D], F32, kind="ExternalInput", name="epsrow", uniquify=False)
            invc2_d = dram.tile([P, 1], F32, kind="ExternalInput", name="invc2", uniquify=False)
            partials_d = dram.tile([P, 8], F32, kind="ExternalOutput", name="partials", uniquify=False)

            # --- constants resident in SBUF
            ident_f = sb_const.tile([P, P], F32, name="ident_f", uniquify=False)
            make_identity(nc, ident_f[:])
            ident = sb_const.tile([P, P], F32R, name="ident", uniquify=False)
            nc.scalar.copy(ident[:], ident_f[:])
            b.ident = ident
            epsrow = sb_const.tile([P, MB, D], F32, name="epsrow_sb", uniquify=False)
            nc.sync.dma_start(out=epsrow[:], in_=epsrow_d[:].rearrange("(m p) n -> p m n", p=P))
            b.epsrow = epsrow
            invc2 = sb_const.tile([P, 1], F32, name="invc2_sb", uniquify=False)
            nc.sync.dma_start(out=invc2[:], in_=invc2_d[:])
            part = sb_const.tile([P, 8], F32, name="part_sb", uniquify=False)
            b.part = part

            for _rep in range(repeat):
                _emit_pipeline(b, nc, sched1, sched2, a1col, a1row, ctcol,
                               epsrow, invc2, partials_d)

    legalize_single_wait(nc)
    return nc


def _emit_pipeline(b, nc, sched1, sched2, a1col, a1row, ctcol, epsrow, invc2,
                   partials_d):
    if True:
        if True:
            # --- NS1 on A1 (uploaded: Cp/c1 + eps I)
            a1c_sb = _load_lhsT(b, a1col[:])
            a1r_sb = b.sb.tile([P, MB, D], F32, tag="ostag", name="a1r_sb", bufs=_TAG_BUFS["ostag"])
            nc.sync.dma_start(out=a1r_sb[:], in_=a1row[:].rearrange("(m p) n -> p m n", p=P))
            st1 = _ns_chain(b, a1c_sb, a1r_sb, sched1, "n1")

            # --- NS1 half-step: S = Y*(1.5 I - 0.5 Z Y)
            lh_z = _get_lhsT(b, st1, "Z")
            lh_y = _get_lhsT(b, st1, "Y")
            tp_stag = _mm_shard(b, lh_z, _get_stream(b, st1, "Y"), -0.5, 1.5 / EPS,
                                tag="tstag")
            tp_full, _ = _bounce_and_gather(b, tp_stag, False, "half")
            s_stag = _mm_shard(b, lh_y, _stream_view(tp_full), 1.0, None)
            s_full, s_a2a = _bounce_and_gather(b, s_stag, True, "sfin")

            # --- middle: V = (Ct @ S)/c2 ; A2 = S @ V + eps I
            ct_sb = _load_lhsT(b, ctcol[:])
            v_stag = _mm_shard(b, ct_sb, _stream_view(s_full), invc2[:, 0:1],
                               None, tag="tstag")
            v_full, _ = _bounce_and_gather(b, v_stag, False, "vmid")
            lh_s = _load_lhsT(b, s_a2a)
            a2_stag = _mm_shard(b, lh_s, _stream_view(v_full), 1.0, 1.0)
            # A2: only A2A needed (lhsT for NS2 iter1); row slab is local staging
            a2a_in = b.dram.tile([NC, SH, SH], F32R, name=b.u("ai_a2"), tag="d_ai", bufs=4)
            for j in range(NC):
                nc.gpsimd.dma_start(
                    out=a2a_in[j].rearrange("(m p) n -> p m n", p=P),
                    in_=a2_stag[:, :, j * SH:(j + 1) * SH])
            a2_a2a = b.dram.tile([NC * SH, SH], F32R, name=b.u("ao_a2"), tag="d_ao", bufs=4)
            nc.gpsimd.collective_compute(
                "AllToAll", ALU.bypass, replica_groups=[list(range(NC))],
                ins=[a2a_in[:]], outs=[a2_a2a[:]],
            )
            a2c_sb = _load_lhsT(b, a2_a2a[:])

            # --- NS2
            st2 = _ns_chain(b, a2c_sb, a2_stag, sched2, "n2")

            # --- trace stage: U2 = Y2 @ Z2 (staging only)
            lh_y2 = _get_lhsT(b, st2, "Y")
            u2_stag = _mm_shard(b, lh_y2, _get_stream(b, st2, "Z"), 1.0, None,
                                tag="tstag")
            y2_stag = st2["Y_stag"]
            part = b.part
            nc.gpsimd.memset(part[:], 0.0)
            tmp = b.sb.tile([P, MB, D], F32, tag="f32tmp", name=b.u("tmp"), bufs=1)
            for m in range(MB):
                nc.vector.tensor_mul(
                    tmp[:, m, :], y2_stag[:, m, :].bitcast(F32),
                    u2_stag[:, m, :].bitcast(F32))
                nc.vector.tensor_reduce(
                    part[:, m:m + 1], tmp[:, m, :], mybir.AxisListType.X, ALU.add)
                nc.vector.tensor_mul(
                    tmp[:, m, :], y2_stag[:, m, :].bitcast(F32), epsrow[:, m, :])
                nc.vector.tensor_reduce(
                    part[:, 2 + m:3 + m], tmp[:, m, :], mybir.AxisListType.X, ALU.add)
            nc.sync.dma_start(out=partials_d[:], in_=part[:])


# ----------------------------------------------------------------------------
# host golden model (mirrors device pipeline exactly, fp32, no hw noise)
def golden_loss(predictions, targets, k1=K1, k2=K2):
    mu_p, Cp = _unpack_row(predictions[0])
    mu_t, Ct = _unpack_row(targets[0])
    c1 = _power_iter_sym(Cp) * 1.02
    c2 = _power_iter_prod(Cp, Ct) * 1.05 / c1
    I = np.eye(D, dtype=np.float32)
    A1 = (Cp / c1 + EPS * I).astype(np.float32)

    def chain(A, sched):
        al, be = sched[0]
        T0 = (al * I + be * A).astype(np.float32)
        Y, Z = A @ T0, T0
        for alk, bek in sched[1:]:
            Pm = Z @ Y
            T = alk * I + bek * Pm
            Y, Z = Y @ T, T @ Z
        return Y, Z

    Y1, Z1 = chain(A1, make_schedule(EPS, 1.0 + EPS, k1))
    S = Y1 @ (1.5 * I - 0.5 * (Z1 @ Y1))
    V = (Ct @ S) / c2
    A2 = (S @ V + EPS * I).astype(np.float32)
    Y2, Z2 = chain(A2, make_schedule(EPS, 1.0 + EPS, k2))
    U2 = Y2 @ Z2
    tr_corr = 1.5 * np.trace(Y2.astype(np.float64)) - 0.5 * float(
        np.sum(Y2.astype(np.float64) * U2.astype(np.float64)))
    tr_sqrtM = np.sqrt(c1 * c2) * tr_corr
    mu_term = float(np.mean((mu_p - mu_t) ** 2))
    return np.float32(mu_term + np.trace(Cp.astype(np.float64))
                      + np.trace(Ct.astype(np.float64)) + 2.0 * tr_sqrtM)


# ----------------------------------------------------------------------------
# entry point
def _get_program():
    key = (K1, K2)
    if key not in _BUILD_CACHE:
        _BUILD_CACHE[key] = build_device_program(K1, K2)
    return _BUILD_CACHE[key]


def kernel(predictions, targets):
    predictions = np.asarray(predictions)
    targets = np.asarray(targets)
    mu_p, Cp = _unpack_row(predictions[0])
    mu_t, Ct = _unpack_row(targets[0])

    c1 = _power_iter_sym(Cp) * 1.02
    c2 = _power_iter_prod(Cp, Ct) * 1.05 / c1

    I = np.eye(D, dtype=np.float32)
    A1 = (Cp / c1).astype(np.float32)
    A1[np.arange(D), np.arange(D)] += EPS

    nc = _get_program()

    in_maps = []
    for c in range(NC):
        sl = slice(c * SH, (c + 1) * SH)
        eps_row = np.zeros((SH, D), np.float32)
        eps_row[np.arange(SH), np.arange(c * SH, (c + 1) * SH)] = EPS
        in_maps.append({
            "a1col": np.ascontiguousarray(A1[:, sl]),
            "a1row": np.ascontiguousarray(A1[sl, :]),
            "ctcol": np.ascontiguousarray(Ct[:, sl]),
            "epsrow": eps_row,
            "invc2": np.full((P, 1), 1.0 / c2, np.float32),
        })

    res = run_bass_kernel_spmd(nc, in_maps, core_ids=list(range(NC)))
    parts = np.stack([r["partials"] for r in res.results])  # [NC, P, 8]
    syu = float(parts[:, :, 0:2].sum(dtype=np.float64))
    trY2 = float(parts[:, :, 2:4].sum(dtype=np.float64)) / EPS
    tr_corr = 1.5 * trY2 - 0.5 * syu
    tr_sqrtM = np.sqrt(c1 * c2) * tr_corr

    mu_term = float(np.mean((mu_p - mu_t) ** 2))
    loss = (mu_term + float(np.trace(Cp.astype(np.float64)))
            + float(np.trace(Ct.astype(np.float64))) + 2.0 * tr_sqrtM)
    return np.float32(loss)



# revision 6
# speedup vs baseline: 4.0977x; 4.0977x over previous
"""Trainium2 Bass kernel for nn_CustomLoss (2-Wasserstein-style Gaussian loss).

loss = mean((mu_p-mu_t)^2) + tr(Cp) + tr(Ct) + 2*tr(sqrtm(S2 @ Ct @ S2)),
       S2 = sqrtm(Cp),  d = 2048, packed inputs (4, 2100224), row 0 used.

Device algorithm: two scaled coupled Newton-Schulz sqrt chains in fp32r
(TensorEngine full rate), 8-way row-sharded across the NeuronCores with
AllGather (full operands for streaming) + AllToAll (column-slice delivery for
the stationary operand, avoiding core-dependent addressing in the SPMD
program). Scalar normalizers and the per-iteration scaling schedule are
host-side; the schedule is input-independent so one NEFF serves all inputs.

Dispatch path: the jitted PJRT executable is built once per process and
cached; per call only the input matrices move host->device. A1 ships as
double-bf16 (base + bf16 residual, fp32-like accuracy at half the bytes),
Ct as single bf16 (it enters the pipeline linearly, once). Column-slice
(lhsT) operands are produced on device by PE transposes of the row slabs
(A1 and Ct are symmetric), and the eps*I row slab is input-independent so
it is uploaded once and kept device-resident across calls.
"""
import numpy as np
import ml_dtypes

import concourse.bass as bass
import concourse.mybir as mybir
import concourse.tile as tile
from concourse.masks import make_identity

# Disable the walrus-embedded BIR simulator: ~4x faster NEFF compiles.
import concourse.bass_utils as _bu
if not getattr(_bu, "_nobirsim_patched", False):
    _orig_bvo = _bu.bir_verify_and_optimise

    def _bvo_fast(tmpdir, inp="bir.json", outp="file.neff", arch=None, *, dve_root=None):
        orig_run = _bu.run_command

        def patched_run(argv, **kw):
            argv = [a.replace("--enable-birsim=true", "--enable-birsim=false")
                    if isinstance(a, str) else a for a in argv]
            return orig_run(argv, **kw)

        _bu.run_command = patched_run
        try:
            return _orig_bvo(tmpdir, inp, outp, arch, dve_root=dve_root)
        finally:
            _bu.run_command = orig_run

    _bu.bir_verify_and_optimise = _bvo_fast
    _bu._nobirsim_patched = True

# ----------------------------------------------------------------------------
# config
D = 2048
NC = 8
SH = D // NC          # 256 rows per core
P = 128
KT = D // P           # 16 k-tiles
MB = SH // P          # 2 m-blocks per shard
NB = D // 512         # 4 n-blocks
CH = 2                # k-tiles per stream chunk
_TAG_BUFS = {"ostag": 2, "tstag": 2, "zstag": 1, "rstream": 2, "lhsT": 3}
EPS = 1e-4            # ridge (normalized units)
QCAP = 2.5            # max scaled eigenvalue (stability margin)
K1 = 9                # NS1 iterations (incl. cheap iter 1) + half-step
K2 = 10               # NS2 iterations (incl. cheap iter 1) + trace correction
F32 = mybir.dt.float32
F32R = mybir.dt.float32r
BF16 = mybir.dt.bfloat16
AF = mybir.ActivationFunctionType
ALU = mybir.AluOpType
BF16NP = ml_dtypes.bfloat16

_RUN_CACHE = {}


# ----------------------------------------------------------------------------
# host: schedule
def _f(q):
    return q * (3.0 - q) ** 2 / 4.0


def _balance_s(a, b, qcap):
    """s with f(s*a) = f(s*b), s*b <= qcap, via bisection."""
    s_hi = min(qcap, 2.9999) / b
    g = lambda s: _f(s * a) - _f(s * b)
    if g(s_hi) <= 0:
        return s_hi
    lo, hi = 1e-12, s_hi
    for _ in range(80):
        mid = 0.5 * (lo + hi)
        if g(mid) > 0:
            hi = mid
        else:
            lo = mid
    return 0.5 * (lo + hi)


def make_schedule(delta, b0, iters, qcap=QCAP):
    a, b = delta, b0
    out = []
    for _ in range(iters):
        s = 1.0 if a > 0.99 * b else _balance_s(a, b, qcap)
        mu = np.sqrt(s)
        out.append((1.5 * mu, -0.5 * mu ** 3))   # (alpha, beta): T = a*I + b*P
        qa, qb = s * a, s * b
        vals = [_f(qa), _f(qb)]
        b = 1.0 if qa <= 1.0 <= qb else max(vals)
        a = min(vals)
    return out


# ----------------------------------------------------------------------------
# host: input prep
_TRI_OFF = np.concatenate([[0], np.cumsum(D - np.arange(D))]).astype(np.int64)


def _unpack_row(v):
    """Packed row -> (mu[D] f64, C[D,D] f32 symmetric). Row-slice loop +
    transpose-add: ~5x faster than a triu fancy-index scatter."""
    mu = np.asarray(v[:D], dtype=np.float64)
    tri = np.asarray(v[D:], dtype=np.float32)
    U = np.zeros((D, D), np.float32)
    off = _TRI_OFF
    for i in range(D):
        U[i, i:] = tri[off[i]:off[i + 1]]
    d = np.diagonal(U).copy()
    C = U + U.T
    np.fill_diagonal(C, d)
    return mu, C


def _power_iter_sym(C, iters=60):
    rng = np.random.default_rng(12345)
    x = rng.standard_normal(D).astype(np.float32)
    lam = 1.0
    for _ in range(iters):
        y = C @ x
        lam = np.linalg.norm(y)
        x = y / lam
    return float(lam)


def _power_iter_prod(Cp, Ct, iters=60):
    rng = np.random.default_rng(54321)
    x = rng.standard_normal(D).astype(np.float32)
    lam = 1.0
    for _ in range(iters):
        y = Cp @ (Ct @ x)
        lam = np.linalg.norm(y)
        x = y / lam
    return float(lam)


# ----------------------------------------------------------------------------
# walrus workaround: this build allows only ONE sync-wait per instruction
class PatchedTileContext(tile.TileContext):
    def _drain_and_barrier(self, tick_clock, wait_clock):
        from concourse.vector_clock import ScopedClock

        probe = self.nc.sync.nop(nofuse=True)
        wait_clock.add_sem_waits(
            probe.ins, ScopedClock({None: tick_clock.global_clock})
        )
        si = probe.ins.sync_info
        waits = list(si.on_wait) if si is not None else []
        if len(waits) > 1:
            si.on_wait = [waits[0]]
            for w in waits[1:]:
                n2 = self.nc.sync.nop(nofuse=True)
                si2 = n2.ins.sync_info
                if si2 is None:
                    n2.ins.sync_info = mybir.SyncInfo(on_wait=[w], on_update=[])
                else:
                    si2.on_wait = [w]
        self.nc.sync.drain()
        self.nc.all_engine_barrier()
        assert self.sems is not None
        popped = self.nc._tile_sem_poison_stack.pop()
        assert popped is self._sem_poison
        self.nc.clear_and_free_semaphores(list(self.sems.allocated().values()))
        self.nc.all_engine_barrier()


def legalize_single_wait(nc):
    uid = 0
    for fn in nc.m.functions:
        for blk in fn.blocks:
            il = blk.instructions
            if not any(
                i.sync_info is not None and len(i.sync_info.on_wait) > 1 for i in il
            ):
                continue
            new = []
            for ins in il:
                si = ins.sync_info
                waits = list(si.on_wait) if si is not None else []
                if len(waits) > 1:
                    si.on_wait = [waits[-1]]
                    for w in waits[:-1]:
                        nop = mybir.InstNoOp(
                            name=f"legalize-wait-{uid}",
                            engine=ins.engine,
                            sync_info=mybir.SyncInfo(on_wait=[w], on_update=[]),
                        )
                        uid += 1
                        new.append(nop)
                new.append(ins)
            blk.instructions = new


# ----------------------------------------------------------------------------
# device program builder
class _B:
    """Builder state."""

    def __init__(self, nc, tc, dram, sb, psum):
        self.nc, self.tc = nc, tc
        self.dram, self.sb, self.psum = dram, sb, psum
        self.uid = 0
        self.ident = None    # [P, P] identity f32
        self.epsrow = None   # [P, MB, D] eps*I row slab (per-core input)

    def u(self, s):
        self.uid += 1
        return f"{s}_{self.uid}"


def _stream_view(full_ap):
    """[D, D] dram AP -> [P, NCH, CH, D] chunked k-tile stream view."""
    return full_ap.rearrange("(ch kb p) n -> p ch kb n", p=P, kb=CH)


def _lhsT_view(a2a_ap):
    """[D, SH] dram AP (A2A out, flat) -> [P, KT, SH]."""
    return a2a_ap.rearrange("(k p) m -> p k m", p=P)


def _mm_shard(b: _B, lhsT_sb, rhs_chunks, scale, eps_coef, tag="ostag"):
    """out_stag[P, MB, D] = (lhsT^T @ rhs) * scale (+ eps_coef * epsrow).

    lhsT_sb: [P, KT, SH] f32 sbuf; rhs_chunks: [P, NCHUNK, CH, D] dram view.
    scale: float or AP. eps_coef: None or float g (adds g * epsrow).
    """
    nc = b.nc
    stag = b.sb.tile([P, MB, D], F32R, tag=tag, name=b.u(tag), bufs=_TAG_BUFS[tag])
    ps = [
        b.psum.tile([P, 512], F32, tag="mmps", name=b.u("ps"))
        for _ in range(MB * NB)
    ]
    for ch in range(KT // CH):
        rt = b.sb.tile([P, CH, D], F32R, tag="rstream", name=b.u("rt"), bufs=_TAG_BUFS["rstream"])
        nc.sync.dma_start(out=rt[:], in_=rhs_chunks[:, ch])
        for kk in range(CH):
            k = ch * CH + kk
            for m in range(MB):
                for n in range(NB):
                    nc.tensor.matmul(
                        ps[m * NB + n][:],
                        lhsT_sb[:, k, m * P:(m + 1) * P],
                        rt[:, kk, n * 512:(n + 1) * 512],
                        start=(k == 0),
                        stop=(k == KT - 1),
                    )
    for m in range(MB):
        for n in range(NB):
            if eps_coef is not None:
                # add (eps_coef/scale) * epsrow into psum pre-eviction so the
                # scaled eviction yields  scale*psum + eps_coef*epsrow
                nc.vector.scalar_tensor_tensor(
                    ps[m * NB + n][:],
                    b.epsrow[:, m, n * 512:(n + 1) * 512],
                    float(eps_coef) / _scale_const(scale),
                    ps[m * NB + n][:],
                    ALU.mult,
                    ALU.add,
                )
            nc.scalar.activation(
                stag[:, m, n * 512:(n + 1) * 512],
                ps[m * NB + n][:],
                AF.Copy,
                scale=scale,
            )
    return stag


def _scale_const(scale):
    assert isinstance(scale, (int, float)), "eps_coef requires constant scale"
    return float(scale)


def _transpose_shard(b: _B, stag):
    """[P, MB, D] staging (rows shard of X) -> [P, KT, SH] = X^T[:, shard cols]."""
    nc = b.nc
    tt = b.sb.tile([P, KT, SH], F32R, tag="lhsT", name=b.u("tt"), bufs=_TAG_BUFS["lhsT"])
    for k in range(KT):
        for m in range(MB):
            tp = b.psum.tile([P, 512], F32R, tag="mmps", name=b.u("tps"))
            nc.tensor.transpose(
                tp[:, 0:P], stag[:, m, k * P:(k + 1) * P], b.ident[:]
            )
            nc.scalar.copy(tt[:, k, m * P:(m + 1) * P], tp[:, 0:P])
    return tt


def _load_lhsT(b: _B, dram_flat_ap):
    """DMA [D, SH] dram -> [P, KT, SH] sbuf."""
    t = b.sb.tile([P, KT, SH], F32R, tag="lhsT", name=b.u("lh"), bufs=_TAG_BUFS["lhsT"])
    b.nc.sync.dma_start(out=t[:], in_=_lhsT_view(dram_flat_ap))
    return t


def _bounce_and_gather(b: _B, stag, want_a2a, name):
    """Write staging to DRAM, AllGather full (+ optionally AllToAll col-slice).

    Returns (full_dram_ap [D, D], a2a_out_ap [D, SH] or None).
    """
    nc = b.nc
    bounce = b.dram.tile([SH, D], F32R, name=b.u(f"bn_{name}"), tag="d_bn", bufs=4)
    nc.gpsimd.dma_start(
        out=bounce[:].rearrange("(m p) n -> p m n", p=P), in_=stag[:]
    )
    full = b.dram.tile([D, D], F32R, name=b.u(f"fl_{name}"), addr_space="Shared", tag="d_fl", bufs=4)
    nc.gpsimd.collective_compute(
        "AllGather",
        ALU.bypass,
        replica_groups=[list(range(NC))],
        ins=[bounce[:]],
        outs=[full[:]],
    )
    a2a_out = None
    if want_a2a:
        a2a_in = b.dram.tile([NC, SH, SH], F32R, name=b.u(f"ai_{name}"), tag="d_ai", bufs=4)
        for j in range(NC):
            nc.gpsimd.dma_start(
                out=a2a_in[j].rearrange("(m p) n -> p m n", p=P),
                in_=stag[:, :, j * SH:(j + 1) * SH],
            )
        a2a_out = b.dram.tile([NC * SH, SH], F32R, name=b.u(f"ao_{name}"), tag="d_ao", bufs=4)
        nc.gpsimd.collective_compute(
            "AllToAll",
            ALU.bypass,
            replica_groups=[list(range(NC))],
            ins=[a2a_in[:]],
            outs=[a2a_out[:]],
        )
    return full[:], (a2a_out[:] if a2a_out is not None else None)


def _ns_chain(b: _B, a_col_lhsT_sb, a_row_stag, sched, name):
    """Run a scaled NS chain. Inputs:
      a_col_lhsT_sb: [P, KT, SH] sbuf = A[:, shard cols]  (lhsT of A)
      a_row_stag:    [P, MB, D] sbuf = A[shard rows, :]   (row slab of A)
    Returns dict with Yfull, Zfull (dram APs), Y_a2a, Z_a2a, Y_stag (sbuf).
    """
    nc = b.nc
    al0, be0 = sched[0]
    # iter 1: T0 = al0*I + be0*A (sharded, elementwise); Z1 = T0; Y1 = A @ T0
    t0f = b.sb.tile([P, MB, D], F32, tag="f32tmp", name=b.u("t0f"), bufs=1)
    t0 = b.sb.tile([P, MB, D], F32R, tag="ostag", name=b.u("t0"), bufs=_TAG_BUFS["ostag"])
    for m in range(MB):
        nc.scalar.mul(t0f[:, m, :], a_row_stag[:, m, :].bitcast(F32), float(be0))
        nc.vector.scalar_tensor_tensor(
            t0f[:, m, :], b.epsrow[:, m, :], float(al0 / EPS),
            t0f[:, m, :], ALU.mult, ALU.add,
        )
        nc.scalar.copy(t0[:, m, :], t0f[:, m, :])
    t0_full, t0_a2a = _bounce_and_gather(b, t0, True, f"{name}t0")
    y_stag = _mm_shard(b, a_col_lhsT_sb, _stream_view(t0_full), 1.0, None)
    y_full, y_a2a = _bounce_and_gather(b, y_stag, True, f"{name}y1")
    st = dict(Yfull=y_full, Y_a2a=y_a2a, Zfull=t0_full, Z_a2a=t0_a2a, Y_stag=y_stag)

    for it in range(1, len(sched)):
        al, be = sched[it]
        lh_z = _get_lhsT(b, st, "Z")
        lh_y = _get_lhsT(b, st, "Y")
        # P = Z @ Y ; T = al*I + be*P  (keep T staging for local transpose)
        t_stag = _mm_shard(b, lh_z, _get_stream(b, st, "Y"), float(be), al / EPS,
                           tag="tstag")
        t_full, _ = _bounce_and_gather(b, t_stag, False, f"{name}t{it}")
        # Z' = T @ Z : lhsT = T^T[:, shard] = transpose of own T staging
        lh_tt = _transpose_shard(b, t_stag)
        z_stag = _mm_shard(b, lh_tt, _get_stream(b, st, "Z"), 1.0, None,
                           tag="zstag")
        # Y' = Y @ T
        y_stag = _mm_shard(b, lh_y, _stream_view(t_full), 1.0, None)
        # batched gather of (Y', Z')
        bounce = b.dram.tile([2 * SH, D], F32R, name=b.u("bnyz"), tag="d_bnyz", bufs=4)
        nc.gpsimd.dma_start(
            out=bounce[:].rearrange("(t m p) n -> t p m n", t=2, p=P)[0],
            in_=y_stag[:])
        nc.gpsimd.dma_start(
            out=bounce[:].rearrange("(t m p) n -> t p m n", t=2, p=P)[1],
            in_=z_stag[:])
        full = b.dram.tile([NC * 2 * SH, D], F32R, name=b.u("flyz"),
                           addr_space="Shared", tag="d_flyz", bufs=4)
        nc.gpsimd.collective_compute(
            "AllGather", ALU.bypass, replica_groups=[list(range(NC))],
            ins=[bounce[:]], outs=[full[:]],
        )
        a2a_in = b.dram.tile([NC, 2, SH, SH], F32R, name=b.u("aiyz"), tag="d_aiyz", bufs=4)
        for j in range(NC):
            nc.gpsimd.dma_start(
                out=a2a_in[j, 0].rearrange("(m p) n -> p m n", p=P),
                in_=y_stag[:, :, j * SH:(j + 1) * SH])
            nc.gpsimd.dma_start(
                out=a2a_in[j, 1].rearrange("(m p) n -> p m n", p=P),
                in_=z_stag[:, :, j * SH:(j + 1) * SH])
        a2a_out = b.dram.tile([NC, 2, SH, SH], F32R, name=b.u("aoyz"), tag="d_aoyz", bufs=4)
        nc.gpsimd.collective_compute(
            "AllToAll", ALU.bypass, replica_groups=[list(range(NC))],
            ins=[a2a_in[:]], outs=[a2a_out[:]],
        )
        # views: full rows = (c, t, m p); Y = t 0, Z = t 1
        fv = full[:].rearrange("(c t kb p) n -> t p c kb n", t=2, kb=CH, p=P)
        av = a2a_out[:].rearrange("s t (kb p) m -> t p s kb m", kb=CH, p=P)
        st = dict(
            Yfull=fv[0], Zfull=fv[1],           # [P, NC, CH, D] chunk views
            Y_a2a=av[0], Z_a2a=av[1],           # [P, s, kb, SH] 4d lhsT views
            Y_stag=y_stag, Z_stag=z_stag,
            chunked=True,
        )
    return st


def _load_lhsT4(b: _B, view4):
    """DMA [P, s, kb, SH] 4d view -> [P, KT, SH] sbuf (k = s*CH + kb)."""
    t = b.sb.tile([P, KT, SH], F32R, tag="lhsT", name=b.u("lh4"), bufs=_TAG_BUFS["lhsT"])
    for s in range(NC):
        b.nc.sync.dma_start(
            out=t[:, s * CH:(s + 1) * CH, :], in_=view4[:, s]
        )
    return t


def _get_lhsT(b, st, key):
    v = st[f"{key}_a2a"]
    if st.get("chunked"):
        return _load_lhsT4(b, v)
    return _load_lhsT(b, v)


def _get_stream(b, st, key):
    v = st[f"{key}full"]
    if st.get("chunked"):
        return v
    return _stream_view(v)


def _load_qrow(b: _B, base_d, res_d):
    """DMA bf16 row slab(s) and convert to an F32R [P, MB, D] staging tile.

    base_d: [SH, D] bf16 dram; res_d: optional bf16 residual (added in f32)."""
    nc = b.nc
    q = b.sb.tile([P, MB, D], BF16, tag="qin", name=b.u("qin"), bufs=1)
    nc.sync.dma_start(out=q[:], in_=base_d[:].rearrange("(m p) n -> p m n", p=P))
    stag = b.sb.tile([P, MB, D], F32R, tag="ostag", name=b.u("qrow"),
                     bufs=_TAG_BUFS["ostag"])
    if res_d is None:
        for m in range(MB):
            nc.scalar.copy(stag[:, m, :], q[:, m, :])
        return stag
    qf = b.sb.tile([P, MB, D], F32, tag="f32tmp", name=b.u("qf"), bufs=1)
    for m in range(MB):
        nc.scalar.copy(qf[:, m, :], q[:, m, :])
    r = b.sb.tile([P, MB, D], BF16, tag="qin", name=b.u("qres"), bufs=1)
    nc.sync.dma_start(out=r[:], in_=res_d[:].rearrange("(m p) n -> p m n", p=P))
    for m in range(MB):
        # f32 += bf16 residual on DVE, then ACT copy applies f32r rounding
        nc.vector.tensor_add(qf[:, m, :], qf[:, m, :], r[:, m, :])
        nc.scalar.copy(stag[:, m, :], qf[:, m, :])
    return stag


def build_device_program(k1, k2):
    sched1 = make_schedule(EPS, 1.0 + EPS, k1)
    sched2 = make_schedule(EPS, 1.0 + EPS, k2)

    nc = bass.Bass(num_devices=NC)
    with PatchedTileContext(nc) as tc:
        with tc.tile_pool(name="dram", bufs=1, space="DRAM") as dram, \
             tc.tile_pool(name="sb", bufs=1) as sb_const, \
             tc.tile_pool(name="sbw", bufs=3) as sbw, \
             tc.tile_pool(name="psum", bufs=8, space="PSUM") as psum:

            b = _B(nc, tc, dram, sbw, psum)

            # --- inputs (a1 double-bf16, ct single bf16, epsrow resident f32)
            a1q = dram.tile([SH, D], BF16, kind="ExternalInput", name="a1q", uniquify=False)
            a1res = dram.tile([SH, D], BF16, kind="ExternalInput", name="a1res", uniquify=False)
            ctq = dram.tile([SH, D], BF16, kind="ExternalInput", name="ctq", uniquify=False)
            epsrow_d = dram.tile([SH, D], F32, kind="ExternalInput", name="epsrow", uniquify=False)
            invc2_d = dram.tile([P, 1], F32, kind="ExternalInput", name="invc2", uniquify=False)
            partials_d = dram.tile([P, 8], F32, kind="ExternalOutput", name="partials", uniquify=False)

            # --- constants resident in SBUF
            ident_f = sb_const.tile([P, P], F32, name="ident_f", uniquify=False)
            make_identity(nc, ident_f[:])
            ident = sb_const.tile([P, P], F32R, name="ident", uniquify=False)
            nc.scalar.copy(ident[:], ident_f[:])
            b.ident = ident
            epsrow = sb_const.tile([P, MB, D], F32, name="epsrow_sb", uniquify=False)
            nc.sync.dma_start(out=epsrow[:], in_=epsrow_d[:].rearrange("(m p) n -> p m n", p=P))
            b.epsrow = epsrow
            invc2 = sb_const.tile([P, 1], F32, name="invc2_sb", uniquify=False)
            nc.sync.dma_start(out=invc2[:], in_=invc2_d[:])
            part = sb_const.tile([P, 8], F32, name="part_sb", uniquify=False)
            b.part = part

            # --- NS1 on A1 (double-bf16 upload; row slab -> local transpose
            # for the column-slice lhsT since A1 is symmetric)
            a1r_stag = _load_qrow(b, a1q, a1res)
            a1c_sb = _transpose_shard(b, a1r_stag)
            st1 = _ns_chain(b, a1c_sb, a1r_stag, sched1, "n1")

            # --- NS1 half-step: S = Y*(1.5 I - 0.5 Z Y)
            lh_z = _get_lhsT(b, st1, "Z")
            lh_y = _get_lhsT(b, st1, "Y")
            tp_stag = _mm_shard(b, lh_z, _get_stream(b, st1, "Y"), -0.5, 1.5 / EPS,
                                tag="tstag")
            tp_full, _ = _bounce_and_gather(b, tp_stag, False, "half")
            s_stag = _mm_shard(b, lh_y, _stream_view(tp_full), 1.0, None)
            s_full, s_a2a = _bounce_and_gather(b, s_stag, True, "sfin")

            # --- middle: V = (Ct @ S)/c2 ; A2 = S @ V + eps I
            ct_stag = _load_qrow(b, ctq, None)
            ct_sb = _transpose_shard(b, ct_stag)
            v_stag = _mm_shard(b, ct_sb, _stream_view(s_full), invc2[:, 0:1],
                               None, tag="tstag")
            v_full, _ = _bounce_and_gather(b, v_stag, False, "vmid")
            lh_s = _load_lhsT(b, s_a2a)
            a2_stag = _mm_shard(b, lh_s, _stream_view(v_full), 1.0, 1.0)
            # A2: only A2A needed (lhsT for NS2 iter1); row slab is local staging
            a2a_in = b.dram.tile([NC, SH, SH], F32R, name=b.u("ai_a2"), tag="d_ai", bufs=4)
            for j in range(NC):
                nc.gpsimd.dma_start(
                    out=a2a_in[j].rearrange("(m p) n -> p m n", p=P),
                    in_=a2_stag[:, :, j * SH:(j + 1) * SH])
            a2_a2a = b.dram.tile([NC * SH, SH], F32R, name=b.u("ao_a2"), tag="d_ao", bufs=4)
            nc.gpsimd.collective_compute(
                "AllToAll", ALU.bypass, replica_groups=[list(range(NC))],
                ins=[a2a_in[:]], outs=[a2_a2a[:]],
            )
            a2c_sb = _load_lhsT(b, a2_a2a[:])

            # --- NS2
            st2 = _ns_chain(b, a2c_sb, a2_stag, sched2, "n2")

            # --- trace stage: U2 = Y2 @ Z2 (staging only)
            lh_y2 = _get_lhsT(b, st2, "Y")
            u2_stag = _mm_shard(b, lh_y2, _get_stream(b, st2, "Z"), 1.0, None,
                                tag="tstag")
            y2_stag = st2["Y_stag"]
            part = b.part
            nc.gpsimd.memset(part[:], 0.0)
            tmp = b.sb.tile([P, MB, D], F32, tag="f32tmp", name=b.u("tmp"), bufs=1)
            for m in range(MB):
                nc.vector.tensor_mul(
                    tmp[:, m, :], y2_stag[:, m, :].bitcast(F32),
                    u2_stag[:, m, :].bitcast(F32))
                nc.vector.tensor_reduce(
                    part[:, m:m + 1], tmp[:, m, :], mybir.AxisListType.X, ALU.add)
                nc.vector.tensor_mul(
                    tmp[:, m, :], y2_stag[:, m, :].bitcast(F32), epsrow[:, m, :])
                nc.vector.tensor_reduce(
                    part[:, 2 + m:3 + m], tmp[:, m, :], mybir.AxisListType.X, ALU.add)
            nc.sync.dma_start(out=partials_d[:], in_=part[:])

    legalize_single_wait(nc)
    return nc


# ----------------------------------------------------------------------------
# host golden model (mirrors device pipeline exactly, fp32, no hw noise)
def golden_loss(predictions, targets, k1=K1, k2=K2):
    mu_p, Cp = _unpack_row(predictions[0])
    mu_t, Ct = _unpack_row(targets[0])
    c1 = _power_iter_sym(Cp) * 1.02
    c2 = _power_iter_prod(Cp, Ct) * 1.05 / c1
    I = np.eye(D, dtype=np.float32)
    A1 = (Cp / c1 + EPS * I).astype(np.float32)
    q = A1.astype(BF16NP).astype(np.float32)
    A1 = q + (A1 - q).astype(BF16NP).astype(np.float32)
    Ctq = Ct.astype(BF16NP).astype(np.float32)

    def chain(A, sched):
        al, be = sched[0]
        T0 = (al * I + be * A).astype(np.float32)
        Y, Z = A @ T0, T0
        for alk, bek in sched[1:]:
            Pm = Z @ Y
            T = alk * I + bek * Pm
            Y, Z = Y @ T, T @ Z
        return Y, Z

    Y1, Z1 = chain(A1, make_schedule(EPS, 1.0 + EPS, k1))
    S = Y1 @ (1.5 * I - 0.5 * (Z1 @ Y1))
    V = (Ctq @ S) / c2
    A2 = (S @ V + EPS * I).astype(np.float32)
    Y2, Z2 = chain(A2, make_schedule(EPS, 1.0 + EPS, k2))
    U2 = Y2 @ Z2
    tr_corr = 1.5 * np.trace(Y2.astype(np.float64)) - 0.5 * float(
        np.sum(Y2.astype(np.float64) * U2.astype(np.float64)))
    tr_sqrtM = np.sqrt(c1 * c2) * tr_corr
    mu_term = float(np.mean((mu_p - mu_t) ** 2))
    return np.float32(mu_term + np.trace(Cp.astype(np.float64))
                      + np.trace(Ct.astype(np.float64)) + 2.0 * tr_sqrtM)


# ----------------------------------------------------------------------------
# dispatch: build program + jitted sharded executable once per process
def _get_runner():
    key = (K1, K2)
    if key in _RUN_CACHE:
        return _RUN_CACHE[key]

    import jax
    from jax.sharding import Mesh, PartitionSpec, NamedSharding
    from jax.experimental.shard_map import shard_map
    from concourse.bass2jax import (
        _bass_exec_p, partition_id_tensor, install_neuronx_cc_hook)

    nc = build_device_program(K1, K2)
    install_neuronx_cc_hook()

    partition_name = nc.partition_id_tensor.name if nc.partition_id_tensor else None
    in_names, out_names, out_avals, zero_shapes = [], [], [], []
    for alloc in nc.m.functions[0].allocations:
        if not isinstance(alloc, mybir.MemoryLocationSet):
            continue
        name = alloc.memorylocations[0].name
        if alloc.kind == "ExternalInput":
            if name != partition_name:
                in_names.append(name)
        elif alloc.kind == "ExternalOutput":
            shape = tuple(alloc.tensor_shape)
            dtype = mybir.dt.np(alloc.dtype)
            out_names.append(name)
            out_avals.append(jax.core.ShapedArray(shape, dtype))
            zero_shapes.append((shape, dtype))
    n_params = len(in_names)
    n_outs = len(out_avals)
    in_names_full = list(in_names) + out_names
    if partition_name is not None:
        in_names_full.append(partition_name)
    donate = tuple(range(n_params, n_params + n_outs))

    def _body(*args):
        operands = list(args)
        if partition_name is not None:
            operands.append(partition_id_tensor())
        outs = _bass_exec_p.bind(
            *operands,
            out_avals=tuple(out_avals),
            in_names=tuple(in_names_full),
            out_names=tuple(out_names),
            lowering_input_output_aliases=(),
            sim_require_finite=True,
            sim_require_nnan=True,
            nc=nc,
        )
        return tuple(outs)

    devices = jax.devices()[:NC]
    assert len(devices) == NC, f"need {NC} devices, got {len(jax.devices())}"
    mesh = Mesh(np.asarray(devices), ("core",))
    in_specs = (PartitionSpec("core"),) * (n_params + n_outs)
    out_specs = (PartitionSpec("core"),) * len(out_names)
    sharded = jax.jit(
        shard_map(_body, mesh=mesh, in_specs=in_specs, out_specs=out_specs,
                  check_rep=False),
        donate_argnums=donate,
        keep_unused=True,
    )
    shard_spec = NamedSharding(mesh, PartitionSpec("core"))

    # eps*I row slabs are input-independent: upload once, reuse every call.
    epsrow_np = np.zeros((D, D), np.float32)
    np.fill_diagonal(epsrow_np, EPS)
    epsrow_dev = jax.device_put(epsrow_np, shard_spec)

    runner = dict(
        sharded=sharded, in_names=in_names, out_names=out_names,
        zero_shapes=zero_shapes, epsrow=epsrow_dev, n_outs=n_outs,
    )
    _RUN_CACHE[key] = runner
    return runner


def kernel(predictions, targets):
    predictions = np.asarray(predictions)
    targets = np.asarray(targets)
    mu_p, Cp = _unpack_row(predictions[0])
    mu_t, Ct = _unpack_row(targets[0])

    c1 = _power_iter_sym(Cp) * 1.02
    c2 = _power_iter_prod(Cp, Ct) * 1.05 / c1

    tr_cp = float(np.trace(Cp, dtype=np.float64))
    tr_ct = float(np.trace(Ct, dtype=np.float64))

    A1 = Cp * np.float32(1.0 / c1)
    didx = np.arange(D)
    A1[didx, didx] += np.float32(EPS)

    a1q = A1.astype(BF16NP)
    a1res = (A1 - a1q.astype(np.float32)).astype(BF16NP)
    ctq = Ct.astype(BF16NP)
    invc2 = np.full((NC * P, 1), 1.0 / c2, np.float32)

    r = _get_runner()
    feed = {
        "a1q": a1q, "a1res": a1res, "ctq": ctq,
        "epsrow": r["epsrow"], "invc2": invc2,
    }
    args = [feed[n] for n in r["in_names"]]
    zeros = [np.zeros((NC * s[0], *s[1:]), dt) for (s, dt) in r["zero_shapes"]]
    out = r["sharded"](*args, *zeros)
    parts = np.asarray(out[0]).reshape(NC, P, 8)

    syu = float(parts[:, :, 0:2].sum(dtype=np.float64))
    trY2 = float(parts[:, :, 2:4].sum(dtype=np.float64)) / EPS
    tr_corr = 1.5 * trY2 - 0.5 * syu
    tr_sqrtM = np.sqrt(c1 * c2) * tr_corr

    mu_term = float(np.mean((mu_p - mu_t) ** 2))
    loss = mu_term + tr_cp + tr_ct + 2.0 * tr_sqrtM
    return np.float32(loss)


# revision 10
# speedup vs baseline: 5.2396x; 1.2787x over previous
"""Trainium2 Bass kernel for nn_CustomLoss (2-Wasserstein-style Gaussian loss).

loss = mean((mu_p-mu_t)^2) + tr(Cp) + tr(Ct) + 2*tr(sqrtm(S2 @ Ct @ S2)),
       S2 = sqrtm(Cp),  d = 2048, packed inputs (4, 2100224), row 0 used.

Device algorithm: two scaled coupled Newton-Schulz sqrt chains in fp32r
(TensorEngine full rate), 8-way row-sharded across the NeuronCores with
AllGather (full operands for streaming) + AllToAll (column-slice delivery for
the stationary operand, avoiding core-dependent addressing in the SPMD
program). Scalar normalizers and the per-iteration scaling schedule are
host-side; the schedule is input-independent so one NEFF serves all inputs.

Dispatch path: the jitted PJRT executable is built once per process and
cached; per call only the input matrices move host->device. A1 ships as
double-bf16 (base + bf16 residual, fp32-like accuracy at half the bytes),
Ct as single bf16 (it enters the pipeline linearly, once). Column-slice
(lhsT) operands are produced on device by PE transposes of the row slabs
(A1 and Ct are symmetric), and the eps*I row slab is input-independent so
it is uploaded once and kept device-resident across calls.
"""
import numpy as np
import ml_dtypes

import concourse.bass as bass
import concourse.mybir as mybir
import concourse.tile as tile
from concourse.masks import make_identity

# Disable the walrus-embedded BIR simulator: ~4x faster NEFF compiles.
import concourse.bass_utils as _bu
if not getattr(_bu, "_nobirsim_patched", False):
    _orig_bvo = _bu.bir_verify_and_optimise

    def _bvo_fast(tmpdir, inp="bir.json", outp="file.neff", arch=None, *, dve_root=None):
        orig_run = _bu.run_command

        def patched_run(argv, **kw):
            argv = [a.replace("--enable-birsim=true", "--enable-birsim=false")
                    if isinstance(a, str) else a for a in argv]
            return orig_run(argv, **kw)

        _bu.run_command = patched_run
        try:
            return _orig_bvo(tmpdir, inp, outp, arch, dve_root=dve_root)
        finally:
            _bu.run_command = orig_run

    _bu.bir_verify_and_optimise = _bvo_fast
    _bu._nobirsim_patched = True

# ----------------------------------------------------------------------------
# config
D = 2048
NC = 8
SH = D // NC          # 256 rows per core
P = 128
KT = D // P           # 16 k-tiles
MB = SH // P          # 2 m-blocks per shard
NB = D // 512         # 4 n-blocks
CH = 2                # k-tiles per stream chunk
_TAG_BUFS = {"ostag": 2, "tstag": 2, "zstag": 1, "rstream": 2, "lhsT": 3}
EPS = 1e-4            # ridge (normalized units)
QCAP = 2.5            # max scaled eigenvalue (stability margin)
K1 = 9                # NS1 iterations (incl. cheap iter 1) + half-step
K2 = 10               # NS2 iterations (incl. cheap iter 1) + trace correction
F32 = mybir.dt.float32
F32R = mybir.dt.float32r
BF16 = mybir.dt.bfloat16
AF = mybir.ActivationFunctionType
ALU = mybir.AluOpType
BF16NP = ml_dtypes.bfloat16

_RUN_CACHE = {}


# ----------------------------------------------------------------------------
# host: schedule
def _f(q):
    return q * (3.0 - q) ** 2 / 4.0


def _balance_s(a, b, qcap):
    """s with f(s*a) = f(s*b), s*b <= qcap, via bisection."""
    s_hi = min(qcap, 2.9999) / b
    g = lambda s: _f(s * a) - _f(s * b)
    if g(s_hi) <= 0:
        return s_hi
    lo, hi = 1e-12, s_hi
    for _ in range(80):
        mid = 0.5 * (lo + hi)
        if g(mid) > 0:
            hi = mid
        else:
            lo = mid
    return 0.5 * (lo + hi)


def make_schedule(delta, b0, iters, qcap=QCAP):
    a, b = delta, b0
    out = []
    for _ in range(iters):
        s = 1.0 if a > 0.99 * b else _balance_s(a, b, qcap)
        mu = np.sqrt(s)
        out.append((1.5 * mu, -0.5 * mu ** 3))   # (alpha, beta): T = a*I + b*P
        qa, qb = s * a, s * b
        vals = [_f(qa), _f(qb)]
        b = 1.0 if qa <= 1.0 <= qb else max(vals)
        a = min(vals)
    return out


# ----------------------------------------------------------------------------
# host: input prep
_TRI_OFF = np.concatenate([[0], np.cumsum(D - np.arange(D))]).astype(np.int64)


def _unpack_row(v):
    """Packed row -> (mu[D] f64, C[D,D] f32 symmetric). Row-slice loop +
    transpose-add: ~5x faster than a triu fancy-index scatter."""
    mu = np.asarray(v[:D], dtype=np.float64)
    tri = np.asarray(v[D:], dtype=np.float32)
    U = np.zeros((D, D), np.float32)
    off = _TRI_OFF
    for i in range(D):
        U[i, i:] = tri[off[i]:off[i + 1]]
    d = np.diagonal(U).copy()
    C = U + U.T
    np.fill_diagonal(C, d)
    return mu, C


def _power_iter_sym(C, iters=25):
    rng = np.random.default_rng(12345)
    x = rng.standard_normal(D).astype(np.float32)
    lam = 1.0
    for _ in range(iters):
        y = C @ x
        lam = np.linalg.norm(y)
        x = y / lam
    return float(lam)


def _power_iter_prod(Cp, Ct, iters=20):
    rng = np.random.default_rng(54321)
    x = rng.standard_normal(D).astype(np.float32)
    lam = 1.0
    for _ in range(iters):
        y = Cp @ (Ct @ x)
        lam = np.linalg.norm(y)
        x = y / lam
    return float(lam)


# ----------------------------------------------------------------------------
# walrus workaround: this build allows only ONE sync-wait per instruction
class PatchedTileContext(tile.TileContext):
    def _drain_and_barrier(self, tick_clock, wait_clock):
        from concourse.vector_clock import ScopedClock

        probe = self.nc.sync.nop(nofuse=True)
        wait_clock.add_sem_waits(
            probe.ins, ScopedClock({None: tick_clock.global_clock})
        )
        si = probe.ins.sync_info
        waits = list(si.on_wait) if si is not None else []
        if len(waits) > 1:
            si.on_wait = [waits[0]]
            for w in waits[1:]:
                n2 = self.nc.sync.nop(nofuse=True)
                si2 = n2.ins.sync_info
                if si2 is None:
                    n2.ins.sync_info = mybir.SyncInfo(on_wait=[w], on_update=[])
                else:
                    si2.on_wait = [w]
        self.nc.sync.drain()
        self.nc.all_engine_barrier()
        assert self.sems is not None
        popped = self.nc._tile_sem_poison_stack.pop()
        assert popped is self._sem_poison
        self.nc.clear_and_free_semaphores(list(self.sems.allocated().values()))
        self.nc.all_engine_barrier()


def legalize_single_wait(nc):
    uid = 0
    for fn in nc.m.functions:
        for blk in fn.blocks:
            il = blk.instructions
            if not any(
                i.sync_info is not None and len(i.sync_info.on_wait) > 1 for i in il
            ):
                continue
            new = []
            for ins in il:
                si = ins.sync_info
                waits = list(si.on_wait) if si is not None else []
                if len(waits) > 1:
                    si.on_wait = [waits[-1]]
                    for w in waits[:-1]:
                        nop = mybir.InstNoOp(
                            name=f"legalize-wait-{uid}",
                            engine=ins.engine,
                            sync_info=mybir.SyncInfo(on_wait=[w], on_update=[]),
                        )
                        uid += 1
                        new.append(nop)
                new.append(ins)
            blk.instructions = new


# ----------------------------------------------------------------------------
# device program builder
class _B:
    """Builder state."""

    def __init__(self, nc, tc, dram, sb, psum):
        self.nc, self.tc = nc, tc
        self.dram, self.sb, self.psum = dram, sb, psum
        self.uid = 0
        self.ident = None    # [P, P] identity f32
        self.epsrow = None   # [P, MB, D] eps*I row slab (per-core input)

    def u(self, s):
        self.uid += 1
        return f"{s}_{self.uid}"


def _stream_view(full_ap):
    """[D, D] dram AP -> [P, NCH, CH, D] chunked k-tile stream view."""
    return full_ap.rearrange("(ch kb p) n -> p ch kb n", p=P, kb=CH)


def _lhsT_view(a2a_ap):
    """[D, SH] dram AP (A2A out, flat) -> [P, KT, SH]."""
    return a2a_ap.rearrange("(k p) m -> p k m", p=P)


def _mm_shard(b: _B, lhsT_sb, rhs_chunks, scale, eps_coef, tag="ostag"):
    """out_stag[P, MB, D] = (lhsT^T @ rhs) * scale (+ eps_coef * epsrow).

    lhsT_sb: [P, KT, SH] f32 sbuf; rhs_chunks: [P, NCHUNK, CH, D] dram view.
    scale: float or AP. eps_coef: None or float g (adds g * epsrow).
    """
    nc = b.nc
    stag = b.sb.tile([P, MB, D], F32R, tag=tag, name=b.u(tag), bufs=_TAG_BUFS[tag])
    ps = [
        b.psum.tile([P, 512], F32, tag="mmps", name=b.u("ps"))
        for _ in range(MB * NB)
    ]
    for ch in range(KT // CH):
        rt = b.sb.tile([P, CH, D], F32R, tag="rstream", name=b.u("rt"), bufs=_TAG_BUFS["rstream"])
        nc.sync.dma_start(out=rt[:], in_=rhs_chunks[:, ch])
        for kk in range(CH):
            k = ch * CH + kk
            for m in range(MB):
                for n in range(NB):
                    nc.tensor.matmul(
                        ps[m * NB + n][:],
                        lhsT_sb[:, k, m * P:(m + 1) * P],
                        rt[:, kk, n * 512:(n + 1) * 512],
                        start=(k == 0),
                        stop=(k == KT - 1),
                    )
    for m in range(MB):
        for n in range(NB):
            if eps_coef is not None:
                # add (eps_coef/scale) * epsrow into psum pre-eviction so the
                # scaled eviction yields  scale*psum + eps_coef*epsrow
                nc.vector.scalar_tensor_tensor(
                    ps[m * NB + n][:],
                    b.epsrow[:, m, n * 512:(n + 1) * 512],
                    float(eps_coef) / _scale_const(scale),
                    ps[m * NB + n][:],
                    ALU.mult,
                    ALU.add,
                )
            nc.scalar.activation(
                stag[:, m, n * 512:(n + 1) * 512],
                ps[m * NB + n][:],
                AF.Copy,
                scale=scale,
            )
    return stag


def _scale_const(scale):
    assert isinstance(scale, (int, float)), "eps_coef requires constant scale"
    return float(scale)


def _transpose_shard(b: _B, stag):
    """[P, MB, D] staging (rows shard of X) -> [P, KT, SH] = X^T[:, shard cols]."""
    nc = b.nc
    tt = b.sb.tile([P, KT, SH], F32R, tag="lhsT", name=b.u("tt"), bufs=_TAG_BUFS["lhsT"])
    for k in range(KT):
        for m in range(MB):
            tp = b.psum.tile([P, 512], F32R, tag="mmps", name=b.u("tps"))
            nc.tensor.transpose(
                tp[:, 0:P], stag[:, m, k * P:(k + 1) * P], b.ident[:]
            )
            nc.scalar.copy(tt[:, k, m * P:(m + 1) * P], tp[:, 0:P])
    return tt


def _load_lhsT(b: _B, dram_flat_ap):
    """DMA [D, SH] dram -> [P, KT, SH] sbuf."""
    t = b.sb.tile([P, KT, SH], F32R, tag="lhsT", name=b.u("lh"), bufs=_TAG_BUFS["lhsT"])
    b.nc.sync.dma_start(out=t[:], in_=_lhsT_view(dram_flat_ap))
    return t


def _bounce_and_gather(b: _B, stag, want_a2a, name):
    """Write staging to DRAM, AllGather full (+ optionally AllToAll col-slice).

    Returns (full_dram_ap [D, D], a2a_out_ap [D, SH] or None).
    """
    nc = b.nc
    bounce = b.dram.tile([SH, D], F32R, name=b.u(f"bn_{name}"), tag="d_bn", bufs=4)
    nc.gpsimd.dma_start(
        out=bounce[:].rearrange("(m p) n -> p m n", p=P), in_=stag[:]
    )
    full = b.dram.tile([D, D], F32R, name=b.u(f"fl_{name}"), addr_space="Shared", tag="d_fl", bufs=4)
    nc.gpsimd.collective_compute(
        "AllGather",
        ALU.bypass,
        replica_groups=[list(range(NC))],
        ins=[bounce[:]],
        outs=[full[:]],
    )
    a2a_out = None
    if want_a2a:
        a2a_in = b.dram.tile([NC, SH, SH], F32R, name=b.u(f"ai_{name}"), tag="d_ai", bufs=4)
        for j in range(NC):
            nc.gpsimd.dma_start(
                out=a2a_in[j].rearrange("(m p) n -> p m n", p=P),
                in_=stag[:, :, j * SH:(j + 1) * SH],
            )
        a2a_out = b.dram.tile([NC * SH, SH], F32R, name=b.u(f"ao_{name}"), tag="d_ao", bufs=4)
        nc.gpsimd.collective_compute(
            "AllToAll",
            ALU.bypass,
            replica_groups=[list(range(NC))],
            ins=[a2a_in[:]],
            outs=[a2a_out[:]],
        )
    return full[:], (a2a_out[:] if a2a_out is not None else None)


def _ns_chain(b: _B, a_col_lhsT_sb, a_row_stag, sched, name):
    """Run a scaled NS chain. Inputs:
      a_col_lhsT_sb: [P, KT, SH] sbuf = A[:, shard cols]  (lhsT of A)
      a_row_stag:    [P, MB, D] sbuf = A[shard rows, :]   (row slab of A)
    Returns dict with Yfull, Zfull (dram APs), Y_a2a, Z_a2a, Y_stag (sbuf).
    """
    nc = b.nc
    al0, be0 = sched[0]
    # iter 1: T0 = al0*I + be0*A (sharded, elementwise); Z1 = T0; Y1 = A @ T0
    t0f = b.sb.tile([P, MB, D], F32, tag="f32tmp", name=b.u("t0f"), bufs=1)
    t0 = b.sb.tile([P, MB, D], F32R, tag="ostag", name=b.u("t0"), bufs=_TAG_BUFS["ostag"])
    for m in range(MB):
        nc.scalar.mul(t0f[:, m, :], a_row_stag[:, m, :].bitcast(F32), float(be0))
        nc.vector.scalar_tensor_tensor(
            t0f[:, m, :], b.epsrow[:, m, :], float(al0 / EPS),
            t0f[:, m, :], ALU.mult, ALU.add,
        )
        nc.scalar.copy(t0[:, m, :], t0f[:, m, :])
    t0_full, t0_a2a = _bounce_and_gather(b, t0, True, f"{name}t0")
    y_stag = _mm_shard(b, a_col_lhsT_sb, _stream_view(t0_full), 1.0, None)
    y_full, y_a2a = _bounce_and_gather(b, y_stag, True, f"{name}y1")
    st = dict(Yfull=y_full, Y_a2a=y_a2a, Zfull=t0_full, Z_a2a=t0_a2a, Y_stag=y_stag)

    for it in range(1, len(sched)):
        al, be = sched[it]
        lh_z = _get_lhsT(b, st, "Z")
        lh_y = _get_lhsT(b, st, "Y")
        # P = Z @ Y ; T = al*I + be*P  (keep T staging for local transpose)
        t_stag = _mm_shard(b, lh_z, _get_stream(b, st, "Y"), float(be), al / EPS,
                           tag="tstag")
        t_full, _ = _bounce_and_gather(b, t_stag, False, f"{name}t{it}")
        # Z' = T @ Z : lhsT = T^T[:, shard] = transpose of own T staging
        lh_tt = _transpose_shard(b, t_stag)
        z_stag = _mm_shard(b, lh_tt, _get_stream(b, st, "Z"), 1.0, None,
                           tag="zstag")
        # Y' = Y @ T
        y_stag = _mm_shard(b, lh_y, _stream_view(t_full), 1.0, None)
        # batched gather of (Y', Z')
        bounce = b.dram.tile([2 * SH, D], F32R, name=b.u("bnyz"), tag="d_bnyz", bufs=4)
        nc.gpsimd.dma_start(
            out=bounce[:].rearrange("(t m p) n -> t p m n", t=2, p=P)[0],
            in_=y_stag[:])
        nc.gpsimd.dma_start(
            out=bounce[:].rearrange("(t m p) n -> t p m n", t=2, p=P)[1],
            in_=z_stag[:])
        full = b.dram.tile([NC * 2 * SH, D], F32R, name=b.u("flyz"),
                           addr_space="Shared", tag="d_flyz", bufs=4)
        nc.gpsimd.collective_compute(
            "AllGather", ALU.bypass, replica_groups=[list(range(NC))],
            ins=[bounce[:]], outs=[full[:]],
        )
        a2a_in = b.dram.tile([NC, 2, SH, SH], F32R, name=b.u("aiyz"), tag="d_aiyz", bufs=4)
        for j in range(NC):
            nc.gpsimd.dma_start(
                out=a2a_in[j, 0].rearrange("(m p) n -> p m n", p=P),
                in_=y_stag[:, :, j * SH:(j + 1) * SH])
            nc.gpsimd.dma_start(
                out=a2a_in[j, 1].rearrange("(m p) n -> p m n", p=P),
                in_=z_stag[:, :, j * SH:(j + 1) * SH])
        a2a_out = b.dram.tile([NC, 2, SH, SH], F32R, name=b.u("aoyz"), tag="d_aoyz", bufs=4)
        nc.gpsimd.collective_compute(
            "AllToAll", ALU.bypass, replica_groups=[list(range(NC))],
            ins=[a2a_in[:]], outs=[a2a_out[:]],
        )
        # views: full rows = (c, t, m p); Y = t 0, Z = t 1
        fv = full[:].rearrange("(c t kb p) n -> t p c kb n", t=2, kb=CH, p=P)
        av = a2a_out[:].rearrange("s t (kb p) m -> t p s kb m", kb=CH, p=P)
        st = dict(
            Yfull=fv[0], Zfull=fv[1],           # [P, NC, CH, D] chunk views
            Y_a2a=av[0], Z_a2a=av[1],           # [P, s, kb, SH] 4d lhsT views
            Y_stag=y_stag, Z_stag=z_stag,
            chunked=True,
        )
    return st


def _load_lhsT4(b: _B, view4):
    """DMA [P, s, kb, SH] 4d view -> [P, KT, SH] sbuf (k = s*CH + kb)."""
    t = b.sb.tile([P, KT, SH], F32R, tag="lhsT", name=b.u("lh4"), bufs=_TAG_BUFS["lhsT"])
    for s in range(NC):
        b.nc.sync.dma_start(
            out=t[:, s * CH:(s + 1) * CH, :], in_=view4[:, s]
        )
    return t


def _get_lhsT(b, st, key):
    v = st[f"{key}_a2a"]
    if st.get("chunked"):
        return _load_lhsT4(b, v)
    return _load_lhsT(b, v)


def _get_stream(b, st, key):
    v = st[f"{key}full"]
    if st.get("chunked"):
        return v
    return _stream_view(v)


def _load_qrow(b: _B, base_d, res_d):
    """DMA bf16 row slab(s) and convert to an F32R [P, MB, D] staging tile.

    base_d: [SH, D] bf16 dram; res_d: optional bf16 residual (added in f32)."""
    nc = b.nc
    q = b.sb.tile([P, MB, D], BF16, tag="qin", name=b.u("qin"), bufs=1)
    nc.sync.dma_start(out=q[:], in_=base_d[:].rearrange("(m p) n -> p m n", p=P))
    stag = b.sb.tile([P, MB, D], F32R, tag="ostag", name=b.u("qrow"),
                     bufs=_TAG_BUFS["ostag"])
    if res_d is None:
        for m in range(MB):
            nc.scalar.copy(stag[:, m, :], q[:, m, :])
        return stag
    qf = b.sb.tile([P, MB, D], F32, tag="f32tmp", name=b.u("qf"), bufs=1)
    for m in range(MB):
        nc.scalar.copy(qf[:, m, :], q[:, m, :])
    r = b.sb.tile([P, MB, D], BF16, tag="qin", name=b.u("qres"), bufs=1)
    nc.sync.dma_start(out=r[:], in_=res_d[:].rearrange("(m p) n -> p m n", p=P))
    for m in range(MB):
        # f32 += bf16 residual on DVE, then ACT copy applies f32r rounding
        nc.vector.tensor_add(qf[:, m, :], qf[:, m, :], r[:, m, :])
        nc.scalar.copy(stag[:, m, :], qf[:, m, :])
    return stag


def build_device_program(k1, k2):
    sched1 = make_schedule(EPS, 1.0 + EPS, k1)
    sched2 = make_schedule(EPS, 1.0 + EPS, k2)

    nc = bass.Bass(num_devices=NC)
    with PatchedTileContext(nc) as tc:
        with tc.tile_pool(name="dram", bufs=1, space="DRAM") as dram, \
             tc.tile_pool(name="sb", bufs=1) as sb_const, \
             tc.tile_pool(name="sbw", bufs=3) as sbw, \
             tc.tile_pool(name="psum", bufs=8, space="PSUM") as psum:

            b = _B(nc, tc, dram, sbw, psum)

            # --- inputs (a1 double-bf16, ct single bf16, epsrow resident f32)
            a1q = dram.tile([SH, D], BF16, kind="ExternalInput", name="a1q", uniquify=False)
            a1res = dram.tile([SH, D], BF16, kind="ExternalInput", name="a1res", uniquify=False)
            ctq = dram.tile([SH, D], BF16, kind="ExternalInput", name="ctq", uniquify=False)
            epsrow_d = dram.tile([SH, D], F32, kind="ExternalInput", name="epsrow", uniquify=False)
            invc2_d = dram.tile([P, 1], F32, kind="ExternalInput", name="invc2", uniquify=False)
            partials_d = dram.tile([P, 8], F32, kind="ExternalOutput", name="partials", uniquify=False)

            # --- constants resident in SBUF
            ident_f = sb_const.tile([P, P], F32, name="ident_f", uniquify=False)
            make_identity(nc, ident_f[:])
            ident = sb_const.tile([P, P], F32R, name="ident", uniquify=False)
            nc.scalar.copy(ident[:], ident_f[:])
            b.ident = ident
            epsrow = sb_const.tile([P, MB, D], F32, name="epsrow_sb", uniquify=False)
            nc.sync.dma_start(out=epsrow[:], in_=epsrow_d[:].rearrange("(m p) n -> p m n", p=P))
            b.epsrow = epsrow
            invc2 = sb_const.tile([P, 1], F32, name="invc2_sb", uniquify=False)
            nc.sync.dma_start(out=invc2[:], in_=invc2_d[:])
            part = sb_const.tile([P, 8], F32, name="part_sb", uniquify=False)
            b.part = part

            # --- NS1 on A1 (double-bf16 upload; row slab -> local transpose
            # for the column-slice lhsT since A1 is symmetric)
            a1r_stag = _load_qrow(b, a1q, a1res)
            a1c_sb = _transpose_shard(b, a1r_stag)
            st1 = _ns_chain(b, a1c_sb, a1r_stag, sched1, "n1")

            # --- NS1 half-step: S = Y*(1.5 I - 0.5 Z Y)
            lh_z = _get_lhsT(b, st1, "Z")
            lh_y = _get_lhsT(b, st1, "Y")
            tp_stag = _mm_shard(b, lh_z, _get_stream(b, st1, "Y"), -0.5, 1.5 / EPS,
                                tag="tstag")
            tp_full, _ = _bounce_and_gather(b, tp_stag, False, "half")
            s_stag = _mm_shard(b, lh_y, _stream_view(tp_full), 1.0, None)
            s_full, s_a2a = _bounce_and_gather(b, s_stag, True, "sfin")

            # --- middle: V = (Ct @ S)/c2 ; A2 = S @ V + eps I
            ct_stag = _load_qrow(b, ctq, None)
            ct_sb = _transpose_shard(b, ct_stag)
            v_stag = _mm_shard(b, ct_sb, _stream_view(s_full), invc2[:, 0:1],
                               None, tag="tstag")
            v_full, _ = _bounce_and_gather(b, v_stag, False, "vmid")
            lh_s = _load_lhsT(b, s_a2a)
            a2_stag = _mm_shard(b, lh_s, _stream_view(v_full), 1.0, 1.0)
            # A2: only A2A needed (lhsT for NS2 iter1); row slab is local staging
            a2a_in = b.dram.tile([NC, SH, SH], F32R, name=b.u("ai_a2"), tag="d_ai", bufs=4)
            for j in range(NC):
                nc.gpsimd.dma_start(
                    out=a2a_in[j].rearrange("(m p) n -> p m n", p=P),
                    in_=a2_stag[:, :, j * SH:(j + 1) * SH])
            a2_a2a = b.dram.tile([NC * SH, SH], F32R, name=b.u("ao_a2"), tag="d_ao", bufs=4)
            nc.gpsimd.collective_compute(
                "AllToAll", ALU.bypass, replica_groups=[list(range(NC))],
                ins=[a2a_in[:]], outs=[a2_a2a[:]],
            )
            a2c_sb = _load_lhsT(b, a2_a2a[:])

            # --- NS2
            st2 = _ns_chain(b, a2c_sb, a2_stag, sched2, "n2")

            # --- trace stage: U2 = Y2 @ Z2 (staging only)
            lh_y2 = _get_lhsT(b, st2, "Y")
            u2_stag = _mm_shard(b, lh_y2, _get_stream(b, st2, "Z"), 1.0, None,
                                tag="tstag")
            y2_stag = st2["Y_stag"]
            part = b.part
            nc.gpsimd.memset(part[:], 0.0)
            tmp = b.sb.tile([P, MB, D], F32, tag="f32tmp", name=b.u("tmp"), bufs=1)
            for m in range(MB):
                nc.vector.tensor_mul(
                    tmp[:, m, :], y2_stag[:, m, :].bitcast(F32),
                    u2_stag[:, m, :].bitcast(F32))
                nc.vector.tensor_reduce(
                    part[:, m:m + 1], tmp[:, m, :], mybir.AxisListType.X, ALU.add)
                nc.vector.tensor_mul(
                    tmp[:, m, :], y2_stag[:, m, :].bitcast(F32), epsrow[:, m, :])
                nc.vector.tensor_reduce(
                    part[:, 2 + m:3 + m], tmp[:, m, :], mybir.AxisListType.X, ALU.add)
            nc.sync.dma_start(out=partials_d[:], in_=part[:])

    legalize_single_wait(nc)
    return nc


# ----------------------------------------------------------------------------
# host golden model (mirrors device pipeline exactly, fp32, no hw noise)
def golden_loss(predictions, targets, k1=K1, k2=K2):
    mu_p, Cp = _unpack_row(predictions[0])
    mu_t, Ct = _unpack_row(targets[0])
    c1 = _power_iter_sym(Cp) * 1.05
    c2 = _power_iter_prod(Cp, Ct) * 1.10 / c1
    I = np.eye(D, dtype=np.float32)
    A1 = (Cp / c1 + EPS * I).astype(np.float32)
    q = A1.astype(BF16NP).astype(np.float32)
    A1 = q + (A1 - q).astype(BF16NP).astype(np.float32)
    Ctq = Ct.astype(BF16NP).astype(np.float32)

    def chain(A, sched):
        al, be = sched[0]
        T0 = (al * I + be * A).astype(np.float32)
        Y, Z = A @ T0, T0
        for alk, bek in sched[1:]:
            Pm = Z @ Y
            T = alk * I + bek * Pm
            Y, Z = Y @ T, T @ Z
        return Y, Z

    Y1, Z1 = chain(A1, make_schedule(EPS, 1.0 + EPS, k1))
    S = Y1 @ (1.5 * I - 0.5 * (Z1 @ Y1))
    V = (Ctq @ S) / c2
    A2 = (S @ V + EPS * I).astype(np.float32)
    Y2, Z2 = chain(A2, make_schedule(EPS, 1.0 + EPS, k2))
    U2 = Y2 @ Z2
    tr_corr = 1.5 * np.trace(Y2.astype(np.float64)) - 0.5 * float(
        np.sum(Y2.astype(np.float64) * U2.astype(np.float64)))
    tr_sqrtM = np.sqrt(c1 * c2) * tr_corr
    mu_term = float(np.mean((mu_p - mu_t) ** 2))
    return np.float32(mu_term + np.trace(Cp.astype(np.float64))
                      + np.trace(Ct.astype(np.float64)) + 2.0 * tr_sqrtM)


# ----------------------------------------------------------------------------
# dispatch: build program + jitted sharded executable once per process
def _get_runner():
    key = (K1, K2)
    if key in _RUN_CACHE:
        return _RUN_CACHE[key]

    import jax
    from jax.sharding import Mesh, PartitionSpec, NamedSharding
    from jax.experimental.shard_map import shard_map
    from concourse.bass2jax import (
        _bass_exec_p, partition_id_tensor, install_neuronx_cc_hook)

    nc = build_device_program(K1, K2)
    install_neuronx_cc_hook()

    partition_name = nc.partition_id_tensor.name if nc.partition_id_tensor else None
    in_names, out_names, out_avals, zero_shapes = [], [], [], []
    for alloc in nc.m.functions[0].allocations:
        if not isinstance(alloc, mybir.MemoryLocationSet):
            continue
        name = alloc.memorylocations[0].name
        if alloc.kind == "ExternalInput":
            if name != partition_name:
                in_names.append(name)
        elif alloc.kind == "ExternalOutput":
            shape = tuple(alloc.tensor_shape)
            dtype = mybir.dt.np(alloc.dtype)
            out_names.append(name)
            out_avals.append(jax.core.ShapedArray(shape, dtype))
            zero_shapes.append((shape, dtype))
    n_params = len(in_names)
    n_outs = len(out_avals)
    in_names_full = list(in_names) + out_names
    if partition_name is not None:
        in_names_full.append(partition_name)
    donate = tuple(range(n_params, n_params + n_outs))

    def _body(*args):
        operands = list(args)
        if partition_name is not None:
            operands.append(partition_id_tensor())
        outs = _bass_exec_p.bind(
            *operands,
            out_avals=tuple(out_avals),
            in_names=tuple(in_names_full),
            out_names=tuple(out_names),
            lowering_input_output_aliases=(),
            sim_require_finite=True,
            sim_require_nnan=True,
            nc=nc,
        )
        return tuple(outs)

    devices = jax.devices()[:NC]
    assert len(devices) == NC, f"need {NC} devices, got {len(jax.devices())}"
    mesh = Mesh(np.asarray(devices), ("core",))
    in_specs = (PartitionSpec("core"),) * (n_params + n_outs)
    out_specs = (PartitionSpec("core"),) * len(out_names)
    sharded = jax.jit(
        shard_map(_body, mesh=mesh, in_specs=in_specs, out_specs=out_specs,
                  check_rep=False),
        donate_argnums=donate,
        keep_unused=True,
    )
    shard_spec = NamedSharding(mesh, PartitionSpec("core"))

    # eps*I row slabs are input-independent: upload once, reuse every call.
    epsrow_np = np.zeros((D, D), np.float32)
    np.fill_diagonal(epsrow_np, EPS)
    epsrow_dev = jax.device_put(epsrow_np, shard_spec)

    runner = dict(
        sharded=sharded, in_names=in_names, out_names=out_names,
        zero_shapes=zero_shapes, epsrow=epsrow_dev, n_outs=n_outs,
        spec=shard_spec, device_put=jax.device_put,
    )
    _RUN_CACHE[key] = runner
    return runner


def kernel(predictions, targets):
    from concurrent.futures import ThreadPoolExecutor

    predictions = np.asarray(predictions)
    targets = np.asarray(targets)
    r = _get_runner()

    with ThreadPoolExecutor(2) as ex:
        fut_p = ex.submit(_unpack_row, predictions[0])
        fut_t = ex.submit(_unpack_row, targets[0])
        mu_t, Ct = fut_t.result()
        # prefetch Ct to the devices; the wire time overlaps the power
        # iterations and A1 prep below
        ct_dev = r["device_put"](Ct.astype(BF16NP), r["spec"])
        mu_p, Cp = fut_p.result()

    c1 = _power_iter_sym(Cp) * 1.05
    c2 = _power_iter_prod(Cp, Ct) * 1.10 / c1

    tr_cp = float(np.trace(Cp, dtype=np.float64))
    tr_ct = float(np.trace(Ct, dtype=np.float64))

    A1 = Cp * np.float32(1.0 / c1)
    didx = np.arange(D)
    A1[didx, didx] += np.float32(EPS)

    a1q = A1.astype(BF16NP)
    a1res = (A1 - a1q.astype(np.float32)).astype(BF16NP)
    invc2 = np.full((NC * P, 1), 1.0 / c2, np.float32)

    feed = {
        "a1q": a1q, "a1res": a1res, "ctq": ct_dev,
        "epsrow": r["epsrow"], "invc2": invc2,
    }
    args = [feed[n] for n in r["in_names"]]
    zeros = [np.zeros((NC * s[0], *s[1:]), dt) for (s, dt) in r["zero_shapes"]]
    out = r["sharded"](*args, *zeros)
    parts = np.asarray(out[0]).reshape(NC, P, 8)

    syu = float(parts[:, :, 0:2].sum(dtype=np.float64))
    trY2 = float(parts[:, :, 2:4].sum(dtype=np.float64)) / EPS
    tr_corr = 1.5 * trY2 - 0.5 * syu
    tr_sqrtM = np.sqrt(c1 * c2) * tr_corr

    mu_term = float(np.mean((mu_p - mu_t) ** 2))
    loss = mu_term + tr_cp + tr_ct + 2.0 * tr_sqrtM
    return np.float32(loss)


# revision 14
# speedup vs baseline: 5.9968x; 1.1445x over previous
"""Trainium2 Bass kernel for nn_CustomLoss (2-Wasserstein-style Gaussian loss).

loss = mean((mu_p-mu_t)^2) + tr(Cp) + tr(Ct) + 2*tr(sqrtm(S2 @ Ct @ S2)),
       S2 = sqrtm(Cp),  d = 2048, packed inputs (4, 2100224), row 0 used.

Device algorithm: two scaled coupled Newton-Schulz sqrt chains in fp32r
(TensorEngine full rate), 8-way row-sharded across the NeuronCores with
AllGather (full operands for streaming) + AllToAll (column-slice delivery for
the stationary operand, avoiding core-dependent addressing in the SPMD
program). Scalar normalizers and the per-iteration scaling schedule are
host-side; the schedule is input-independent so one NEFF serves all inputs.

Dispatch path: the jitted PJRT executable is built once per process and
cached; per call only the input matrices move host->device. A1 ships as
double-bf16 (base + bf16 residual, fp32-like accuracy at half the bytes),
Ct as single bf16 (it enters the pipeline linearly, once). Column-slice
(lhsT) operands are produced on device by PE transposes of the row slabs
(A1 and Ct are symmetric), and the eps*I row slab is input-independent so
it is uploaded once and kept device-resident across calls.
"""
import numpy as np
import ml_dtypes

import concourse.bass as bass
import concourse.mybir as mybir
import concourse.tile as tile
from concourse.masks import make_identity

# Disable the walrus-embedded BIR simulator: ~4x faster NEFF compiles.
import concourse.bass_utils as _bu
if not getattr(_bu, "_nobirsim_patched", False):
    _orig_bvo = _bu.bir_verify_and_optimise

    def _bvo_fast(tmpdir, inp="bir.json", outp="file.neff", arch=None, *, dve_root=None):
        orig_run = _bu.run_command

        def patched_run(argv, **kw):
            argv = [a.replace("--enable-birsim=true", "--enable-birsim=false")
                    if isinstance(a, str) else a for a in argv]
            return orig_run(argv, **kw)

        _bu.run_command = patched_run
        try:
            return _orig_bvo(tmpdir, inp, outp, arch, dve_root=dve_root)
        finally:
            _bu.run_command = orig_run

    _bu.bir_verify_and_optimise = _bvo_fast
    _bu._nobirsim_patched = True

# ----------------------------------------------------------------------------
# config
D = 2048
NC = 8
SH = D // NC          # 256 rows per core
P = 128
KT = D // P           # 16 k-tiles
MB = SH // P          # 2 m-blocks per shard
NB = D // 512         # 4 n-blocks
CH = 2                # k-tiles per stream chunk
_TAG_BUFS = {"ostag": 2, "tstag": 2, "zstag": 1, "rstream": 2, "lhsT": 3}
EPS = 1e-4            # ridge (normalized units)
QCAP = 2.5            # max scaled eigenvalue (stability margin)
K1 = 9                # NS1 iterations (incl. cheap iter 1) + half-step
K2 = 10               # NS2 iterations (incl. cheap iter 1) + trace correction
F32 = mybir.dt.float32
F32R = mybir.dt.float32r
BF16 = mybir.dt.bfloat16
I8 = mybir.dt.int8
AF = mybir.ActivationFunctionType
ALU = mybir.AluOpType
BF16NP = ml_dtypes.bfloat16

_RUN_CACHE = {}


# ----------------------------------------------------------------------------
# host: schedule
def _f(q):
    return q * (3.0 - q) ** 2 / 4.0


def _balance_s(a, b, qcap):
    """s with f(s*a) = f(s*b), s*b <= qcap, via bisection."""
    s_hi = min(qcap, 2.9999) / b
    g = lambda s: _f(s * a) - _f(s * b)
    if g(s_hi) <= 0:
        return s_hi
    lo, hi = 1e-12, s_hi
    for _ in range(80):
        mid = 0.5 * (lo + hi)
        if g(mid) > 0:
            hi = mid
        else:
            lo = mid
    return 0.5 * (lo + hi)


def make_schedule(delta, b0, iters, qcap=QCAP):
    a, b = delta, b0
    out = []
    for _ in range(iters):
        s = 1.0 if a > 0.99 * b else _balance_s(a, b, qcap)
        mu = np.sqrt(s)
        out.append((1.5 * mu, -0.5 * mu ** 3))   # (alpha, beta): T = a*I + b*P
        qa, qb = s * a, s * b
        vals = [_f(qa), _f(qb)]
        b = 1.0 if qa <= 1.0 <= qb else max(vals)
        a = min(vals)
    return out


# ----------------------------------------------------------------------------
# host: input prep
_TRI_OFF = np.concatenate([[0], np.cumsum(D - np.arange(D))]).astype(np.int64)


def _unpack_row(v):
    """Packed row -> (mu[D] f64, C[D,D] f32 symmetric). Row-slice loop +
    transpose-add: ~5x faster than a triu fancy-index scatter."""
    mu = np.asarray(v[:D], dtype=np.float64)
    tri = np.asarray(v[D:], dtype=np.float32)
    U = np.zeros((D, D), np.float32)
    off = _TRI_OFF
    for i in range(D):
        U[i, i:] = tri[off[i]:off[i + 1]]
    d = np.diagonal(U).copy()
    C = U + U.T
    np.fill_diagonal(C, d)
    return mu, C


def _power_iter_sym(C, iters=25):
    rng = np.random.default_rng(12345)
    x = rng.standard_normal(D).astype(np.float32)
    lam = 1.0
    for _ in range(iters):
        y = C @ x
        lam = np.linalg.norm(y)
        x = y / lam
    return float(lam)


def _power_iter_prod(Cp, Ct, iters=20):
    rng = np.random.default_rng(54321)
    x = rng.standard_normal(D).astype(np.float32)
    lam = 1.0
    for _ in range(iters):
        y = Cp @ (Ct @ x)
        lam = np.linalg.norm(y)
        x = y / lam
    return float(lam)


# ----------------------------------------------------------------------------
# walrus workaround: this build allows only ONE sync-wait per instruction
class PatchedTileContext(tile.TileContext):
    def _drain_and_barrier(self, tick_clock, wait_clock):
        from concourse.vector_clock import ScopedClock

        probe = self.nc.sync.nop(nofuse=True)
        wait_clock.add_sem_waits(
            probe.ins, ScopedClock({None: tick_clock.global_clock})
        )
        si = probe.ins.sync_info
        waits = list(si.on_wait) if si is not None else []
        if len(waits) > 1:
            si.on_wait = [waits[0]]
            for w in waits[1:]:
                n2 = self.nc.sync.nop(nofuse=True)
                si2 = n2.ins.sync_info
                if si2 is None:
                    n2.ins.sync_info = mybir.SyncInfo(on_wait=[w], on_update=[])
                else:
                    si2.on_wait = [w]
        self.nc.sync.drain()
        self.nc.all_engine_barrier()
        assert self.sems is not None
        popped = self.nc._tile_sem_poison_stack.pop()
        assert popped is self._sem_poison
        self.nc.clear_and_free_semaphores(list(self.sems.allocated().values()))
        self.nc.all_engine_barrier()


def legalize_single_wait(nc):
    uid = 0
    for fn in nc.m.functions:
        for blk in fn.blocks:
            il = blk.instructions
            if not any(
                i.sync_info is not None and len(i.sync_info.on_wait) > 1 for i in il
            ):
                continue
            new = []
            for ins in il:
                si = ins.sync_info
                waits = list(si.on_wait) if si is not None else []
                if len(waits) > 1:
                    si.on_wait = [waits[-1]]
                    for w in waits[:-1]:
                        nop = mybir.InstNoOp(
                            name=f"legalize-wait-{uid}",
                            engine=ins.engine,
                            sync_info=mybir.SyncInfo(on_wait=[w], on_update=[]),
                        )
                        uid += 1
                        new.append(nop)
                new.append(ins)
            blk.instructions = new


# ----------------------------------------------------------------------------
# device program builder
class _B:
    """Builder state."""

    def __init__(self, nc, tc, dram, sb, psum):
        self.nc, self.tc = nc, tc
        self.dram, self.sb, self.psum = dram, sb, psum
        self.uid = 0
        self.ident = None    # [P, P] identity f32
        self.epsrow = None   # [P, MB, D] eps*I row slab (per-core input)

    def u(self, s):
        self.uid += 1
        return f"{s}_{self.uid}"


def _stream_view(full_ap):
    """[D, D] dram AP -> [P, NCH, CH, D] chunked k-tile stream view."""
    return full_ap.rearrange("(ch kb p) n -> p ch kb n", p=P, kb=CH)


def _lhsT_view(a2a_ap):
    """[D, SH] dram AP (A2A out, flat) -> [P, KT, SH]."""
    return a2a_ap.rearrange("(k p) m -> p k m", p=P)


def _mm_shard(b: _B, lhsT_sb, rhs_chunks, scale, eps_coef, tag="ostag"):
    """out_stag[P, MB, D] = (lhsT^T @ rhs) * scale (+ eps_coef * epsrow).

    lhsT_sb: [P, KT, SH] f32 sbuf; rhs_chunks: [P, NCHUNK, CH, D] dram view.
    scale: float or AP. eps_coef: None or float g (adds g * epsrow).
    """
    nc = b.nc
    stag = b.sb.tile([P, MB, D], F32R, tag=tag, name=b.u(tag), bufs=_TAG_BUFS[tag])
    ps = [
        b.psum.tile([P, 512], F32, tag="mmps", name=b.u("ps"))
        for _ in range(MB * NB)
    ]
    for ch in range(KT // CH):
        rt = b.sb.tile([P, CH, D], F32R, tag="rstream", name=b.u("rt"), bufs=_TAG_BUFS["rstream"])
        nc.sync.dma_start(out=rt[:], in_=rhs_chunks[:, ch])
        for kk in range(CH):
            k = ch * CH + kk
            for m in range(MB):
                for n in range(NB):
                    nc.tensor.matmul(
                        ps[m * NB + n][:],
                        lhsT_sb[:, k, m * P:(m + 1) * P],
                        rt[:, kk, n * 512:(n + 1) * 512],
                        start=(k == 0),
                        stop=(k == KT - 1),
                    )
    for m in range(MB):
        for n in range(NB):
            if eps_coef is not None:
                # add (eps_coef/scale) * epsrow into psum pre-eviction so the
                # scaled eviction yields  scale*psum + eps_coef*epsrow
                nc.vector.scalar_tensor_tensor(
                    ps[m * NB + n][:],
                    b.epsrow[:, m, n * 512:(n + 1) * 512],
                    float(eps_coef) / _scale_const(scale),
                    ps[m * NB + n][:],
                    ALU.mult,
                    ALU.add,
                )
            nc.scalar.activation(
                stag[:, m, n * 512:(n + 1) * 512],
                ps[m * NB + n][:],
                AF.Copy,
                scale=scale,
            )
    return stag


def _scale_const(scale):
    assert isinstance(scale, (int, float)), "eps_coef requires constant scale"
    return float(scale)


def _transpose_shard(b: _B, stag):
    """[P, MB, D] staging (rows shard of X) -> [P, KT, SH] = X^T[:, shard cols]."""
    nc = b.nc
    tt = b.sb.tile([P, KT, SH], F32R, tag="lhsT", name=b.u("tt"), bufs=_TAG_BUFS["lhsT"])
    for k in range(KT):
        for m in range(MB):
            tp = b.psum.tile([P, 512], F32R, tag="mmps", name=b.u("tps"))
            nc.tensor.transpose(
                tp[:, 0:P], stag[:, m, k * P:(k + 1) * P], b.ident[:]
            )
            nc.scalar.copy(tt[:, k, m * P:(m + 1) * P], tp[:, 0:P])
    return tt


def _load_lhsT(b: _B, dram_flat_ap):
    """DMA [D, SH] dram -> [P, KT, SH] sbuf."""
    t = b.sb.tile([P, KT, SH], F32R, tag="lhsT", name=b.u("lh"), bufs=_TAG_BUFS["lhsT"])
    b.nc.sync.dma_start(out=t[:], in_=_lhsT_view(dram_flat_ap))
    return t


def _bounce_and_gather(b: _B, stag, want_a2a, name):
    """Write staging to DRAM, AllGather full (+ optionally AllToAll col-slice).

    Returns (full_dram_ap [D, D], a2a_out_ap [D, SH] or None).
    """
    nc = b.nc
    bounce = b.dram.tile([SH, D], F32R, name=b.u(f"bn_{name}"), tag="d_bn", bufs=4)
    nc.gpsimd.dma_start(
        out=bounce[:].rearrange("(m p) n -> p m n", p=P), in_=stag[:]
    )
    full = b.dram.tile([D, D], F32R, name=b.u(f"fl_{name}"), addr_space="Shared", tag="d_fl", bufs=4)
    nc.gpsimd.collective_compute(
        "AllGather",
        ALU.bypass,
        replica_groups=[list(range(NC))],
        ins=[bounce[:]],
        outs=[full[:]],
    )
    a2a_out = None
    if want_a2a:
        a2a_in = b.dram.tile([NC, SH, SH], F32R, name=b.u(f"ai_{name}"), tag="d_ai", bufs=4)
        for j in range(NC):
            nc.gpsimd.dma_start(
                out=a2a_in[j].rearrange("(m p) n -> p m n", p=P),
                in_=stag[:, :, j * SH:(j + 1) * SH],
            )
        a2a_out = b.dram.tile([NC * SH, SH], F32R, name=b.u(f"ao_{name}"), tag="d_ao", bufs=4)
        nc.gpsimd.collective_compute(
            "AllToAll",
            ALU.bypass,
            replica_groups=[list(range(NC))],
            ins=[a2a_in[:]],
            outs=[a2a_out[:]],
        )
    return full[:], (a2a_out[:] if a2a_out is not None else None)


def _ns_chain(b: _B, a_col_lhsT_sb, a_row_stag, sched, name):
    """Run a scaled NS chain. Inputs:
      a_col_lhsT_sb: [P, KT, SH] sbuf = A[:, shard cols]  (lhsT of A)
      a_row_stag:    [P, MB, D] sbuf = A[shard rows, :]   (row slab of A)
    Returns dict with Yfull, Zfull (dram APs), Y_a2a, Z_a2a, Y_stag (sbuf).
    """
    nc = b.nc
    al0, be0 = sched[0]
    # iter 1: T0 = al0*I + be0*A (sharded, elementwise); Z1 = T0; Y1 = A @ T0
    t0f = b.sb.tile([P, MB, D], F32, tag="f32tmp", name=b.u("t0f"), bufs=1)
    t0 = b.sb.tile([P, MB, D], F32R, tag="ostag", name=b.u("t0"), bufs=_TAG_BUFS["ostag"])
    for m in range(MB):
        nc.scalar.mul(t0f[:, m, :], a_row_stag[:, m, :].bitcast(F32), float(be0))
        nc.vector.scalar_tensor_tensor(
            t0f[:, m, :], b.epsrow[:, m, :], float(al0 / EPS),
            t0f[:, m, :], ALU.mult, ALU.add,
        )
        nc.scalar.copy(t0[:, m, :], t0f[:, m, :])
    t0_full, t0_a2a = _bounce_and_gather(b, t0, True, f"{name}t0")
    y_stag = _mm_shard(b, a_col_lhsT_sb, _stream_view(t0_full), 1.0, None)
    y_full, y_a2a = _bounce_and_gather(b, y_stag, True, f"{name}y1")
    st = dict(Yfull=y_full, Y_a2a=y_a2a, Zfull=t0_full, Z_a2a=t0_a2a, Y_stag=y_stag)

    for it in range(1, len(sched)):
        al, be = sched[it]
        lh_z = _get_lhsT(b, st, "Z")
        lh_y = _get_lhsT(b, st, "Y")
        # P = Z @ Y ; T = al*I + be*P  (keep T staging for local transpose)
        t_stag = _mm_shard(b, lh_z, _get_stream(b, st, "Y"), float(be), al / EPS,
                           tag="tstag")
        t_full, _ = _bounce_and_gather(b, t_stag, False, f"{name}t{it}")
        # Z' = T @ Z : lhsT = T^T[:, shard] = transpose of own T staging
        lh_tt = _transpose_shard(b, t_stag)
        z_stag = _mm_shard(b, lh_tt, _get_stream(b, st, "Z"), 1.0, None,
                           tag="zstag")
        # Y' = Y @ T
        y_stag = _mm_shard(b, lh_y, _stream_view(t_full), 1.0, None)
        # batched gather of (Y', Z')
        bounce = b.dram.tile([2 * SH, D], F32R, name=b.u("bnyz"), tag="d_bnyz", bufs=4)
        nc.gpsimd.dma_start(
            out=bounce[:].rearrange("(t m p) n -> t p m n", t=2, p=P)[0],
            in_=y_stag[:])
        nc.gpsimd.dma_start(
            out=bounce[:].rearrange("(t m p) n -> t p m n", t=2, p=P)[1],
            in_=z_stag[:])
        full = b.dram.tile([NC * 2 * SH, D], F32R, name=b.u("flyz"),
                           addr_space="Shared", tag="d_flyz", bufs=4)
        nc.gpsimd.collective_compute(
            "AllGather", ALU.bypass, replica_groups=[list(range(NC))],
            ins=[bounce[:]], outs=[full[:]],
        )
        a2a_in = b.dram.tile([NC, 2, SH, SH], F32R, name=b.u("aiyz"), tag="d_aiyz", bufs=4)
        for j in range(NC):
            nc.gpsimd.dma_start(
                out=a2a_in[j, 0].rearrange("(m p) n -> p m n", p=P),
                in_=y_stag[:, :, j * SH:(j + 1) * SH])
            nc.gpsimd.dma_start(
                out=a2a_in[j, 1].rearrange("(m p) n -> p m n", p=P),
                in_=z_stag[:, :, j * SH:(j + 1) * SH])
        a2a_out = b.dram.tile([NC, 2, SH, SH], F32R, name=b.u("aoyz"), tag="d_aoyz", bufs=4)
        nc.gpsimd.collective_compute(
            "AllToAll", ALU.bypass, replica_groups=[list(range(NC))],
            ins=[a2a_in[:]], outs=[a2a_out[:]],
        )
        # views: full rows = (c, t, m p); Y = t 0, Z = t 1
        fv = full[:].rearrange("(c t kb p) n -> t p c kb n", t=2, kb=CH, p=P)
        av = a2a_out[:].rearrange("s t (kb p) m -> t p s kb m", kb=CH, p=P)
        st = dict(
            Yfull=fv[0], Zfull=fv[1],           # [P, NC, CH, D] chunk views
            Y_a2a=av[0], Z_a2a=av[1],           # [P, s, kb, SH] 4d lhsT views
            Y_stag=y_stag, Z_stag=z_stag,
            chunked=True,
        )
    return st


def _load_lhsT4(b: _B, view4):
    """DMA [P, s, kb, SH] 4d view -> [P, KT, SH] sbuf (k = s*CH + kb)."""
    t = b.sb.tile([P, KT, SH], F32R, tag="lhsT", name=b.u("lh4"), bufs=_TAG_BUFS["lhsT"])
    for s in range(NC):
        b.nc.sync.dma_start(
            out=t[:, s * CH:(s + 1) * CH, :], in_=view4[:, s]
        )
    return t


def _get_lhsT(b, st, key):
    v = st[f"{key}_a2a"]
    if st.get("chunked"):
        return _load_lhsT4(b, v)
    return _load_lhsT(b, v)


def _get_stream(b, st, key):
    v = st[f"{key}full"]
    if st.get("chunked"):
        return v
    return _stream_view(v)


def _load_qrow(b: _B, base_d, res_d=None, sc_d=None):
    """DMA a bf16 row slab and convert to an F32R [P, MB, D] staging tile.

    base_d: [SH, D] bf16 dram. res_d/sc_d: optional int8 residual [SH, D]
    plus per-row f32 scales [SH, 1]; the dequantized residual is added in
    f32 before the f32r rounding copy."""
    nc = b.nc
    q = b.sb.tile([P, MB, D], BF16, tag="qin", name=b.u("qin"), bufs=1)
    nc.sync.dma_start(out=q[:], in_=base_d[:].rearrange("(m p) n -> p m n", p=P))
    stag = b.sb.tile([P, MB, D], F32R, tag="ostag", name=b.u("qrow"),
                     bufs=_TAG_BUFS["ostag"])
    if res_d is None:
        for m in range(MB):
            nc.scalar.copy(stag[:, m, :], q[:, m, :])
        return stag
    i8 = b.sb.tile([P, MB, D], I8, tag="qi8", name=b.u("qi8"), bufs=1)
    nc.sync.dma_start(out=i8[:], in_=res_d[:].rearrange("(m p) n -> p m n", p=P))
    sc = b.sb.tile([P, MB, 1], F32, tag="qsc", name=b.u("qsc"), bufs=1)
    nc.sync.dma_start(out=sc[:], in_=sc_d[:].rearrange("(m p) o -> p m o", p=P))
    qf = b.sb.tile([P, MB, D], F32, tag="f32tmp", name=b.u("qf"), bufs=1)
    for m in range(MB):
        nc.scalar.copy(qf[:, m, :], q[:, m, :])
        # qf += rowscale * int8 residual (DVE dequant+add), then ACT copy
        # applies the f32r rounding
        nc.vector.scalar_tensor_tensor(
            qf[:, m, :], i8[:, m, :], sc[:, m, :], qf[:, m, :],
            ALU.mult, ALU.add,
        )
        nc.scalar.copy(stag[:, m, :], qf[:, m, :])
    return stag


def build_device_program(k1, k2):
    sched1 = make_schedule(EPS, 1.0 + EPS, k1)
    sched2 = make_schedule(EPS, 1.0 + EPS, k2)

    nc = bass.Bass(num_devices=NC)
    with PatchedTileContext(nc) as tc:
        with tc.tile_pool(name="dram", bufs=1, space="DRAM") as dram, \
             tc.tile_pool(name="sb", bufs=1) as sb_const, \
             tc.tile_pool(name="sbw", bufs=3) as sbw, \
             tc.tile_pool(name="psum", bufs=8, space="PSUM") as psum:

            b = _B(nc, tc, dram, sbw, psum)

            # --- inputs (a1 double-bf16, ct single bf16, epsrow resident f32)
            a1q = dram.tile([SH, D], BF16, kind="ExternalInput", name="a1q", uniquify=False)
            a1res = dram.tile([SH, D], I8, kind="ExternalInput", name="a1res", uniquify=False)
            a1sc = dram.tile([SH, 1], F32, kind="ExternalInput", name="a1sc", uniquify=False)
            ctq = dram.tile([SH, D], BF16, kind="ExternalInput", name="ctq", uniquify=False)
            epsrow_d = dram.tile([SH, D], F32, kind="ExternalInput", name="epsrow", uniquify=False)
            invc2_d = dram.tile([P, 1], F32, kind="ExternalInput", name="invc2", uniquify=False)
            partials_d = dram.tile([P, 8], F32, kind="ExternalOutput", name="partials", uniquify=False)

            # --- constants resident in SBUF
            ident_f = sb_const.tile([P, P], F32, name="ident_f", uniquify=False)
            make_identity(nc, ident_f[:])
            ident = sb_const.tile([P, P], F32R, name="ident", uniquify=False)
            nc.scalar.copy(ident[:], ident_f[:])
            b.ident = ident
            epsrow = sb_const.tile([P, MB, D], F32, name="epsrow_sb", uniquify=False)
            nc.sync.dma_start(out=epsrow[:], in_=epsrow_d[:].rearrange("(m p) n -> p m n", p=P))
            b.epsrow = epsrow
            invc2 = sb_const.tile([P, 1], F32, name="invc2_sb", uniquify=False)
            nc.sync.dma_start(out=invc2[:], in_=invc2_d[:])
            part = sb_const.tile([P, 8], F32, name="part_sb", uniquify=False)
            b.part = part

            # --- NS1 on A1 (double-bf16 upload; row slab -> local transpose
            # for the column-slice lhsT since A1 is symmetric)
            a1r_stag = _load_qrow(b, a1q, a1res, a1sc)
            a1c_sb = _transpose_shard(b, a1r_stag)
            st1 = _ns_chain(b, a1c_sb, a1r_stag, sched1, "n1")

            # --- NS1 half-step: S = Y*(1.5 I - 0.5 Z Y)
            lh_z = _get_lhsT(b, st1, "Z")
            lh_y = _get_lhsT(b, st1, "Y")
            tp_stag = _mm_shard(b, lh_z, _get_stream(b, st1, "Y"), -0.5, 1.5 / EPS,
                                tag="tstag")
            tp_full, _ = _bounce_and_gather(b, tp_stag, False, "half")
            s_stag = _mm_shard(b, lh_y, _stream_view(tp_full), 1.0, None)
            s_full, s_a2a = _bounce_and_gather(b, s_stag, True, "sfin")

            # --- middle: V = (Ct @ S)/c2 ; A2 = S @ V + eps I
            ct_stag = _load_qrow(b, ctq)
            ct_sb = _transpose_shard(b, ct_stag)
            v_stag = _mm_shard(b, ct_sb, _stream_view(s_full), invc2[:, 0:1],
                               None, tag="tstag")
            v_full, _ = _bounce_and_gather(b, v_stag, False, "vmid")
            lh_s = _load_lhsT(b, s_a2a)
            a2_stag = _mm_shard(b, lh_s, _stream_view(v_full), 1.0, 1.0)
            # A2: only A2A needed (lhsT for NS2 iter1); row slab is local staging
            a2a_in = b.dram.tile([NC, SH, SH], F32R, name=b.u("ai_a2"), tag="d_ai", bufs=4)
            for j in range(NC):
                nc.gpsimd.dma_start(
                    out=a2a_in[j].rearrange("(m p) n -> p m n", p=P),
                    in_=a2_stag[:, :, j * SH:(j + 1) * SH])
            a2_a2a = b.dram.tile([NC * SH, SH], F32R, name=b.u("ao_a2"), tag="d_ao", bufs=4)
            nc.gpsimd.collective_compute(
                "AllToAll", ALU.bypass, replica_groups=[list(range(NC))],
                ins=[a2a_in[:]], outs=[a2_a2a[:]],
            )
            a2c_sb = _load_lhsT(b, a2_a2a[:])

            # --- NS2
            st2 = _ns_chain(b, a2c_sb, a2_stag, sched2, "n2")

            # --- trace stage: U2 = Y2 @ Z2 (staging only)
            lh_y2 = _get_lhsT(b, st2, "Y")
            u2_stag = _mm_shard(b, lh_y2, _get_stream(b, st2, "Z"), 1.0, None,
                                tag="tstag")
            y2_stag = st2["Y_stag"]
            part = b.part
            nc.gpsimd.memset(part[:], 0.0)
            tmp = b.sb.tile([P, MB, D], F32, tag="f32tmp", name=b.u("tmp"), bufs=1)
            for m in range(MB):
                nc.vector.tensor_mul(
                    tmp[:, m, :], y2_stag[:, m, :].bitcast(F32),
                    u2_stag[:, m, :].bitcast(F32))
                nc.vector.tensor_reduce(
                    part[:, m:m + 1], tmp[:, m, :], mybir.AxisListType.X, ALU.add)
                nc.vector.tensor_mul(
                    tmp[:, m, :], y2_stag[:, m, :].bitcast(F32), epsrow[:, m, :])
                nc.vector.tensor_reduce(
                    part[:, 2 + m:3 + m], tmp[:, m, :], mybir.AxisListType.X, ALU.add)
            nc.sync.dma_start(out=partials_d[:], in_=part[:])

    legalize_single_wait(nc)
    return nc


# ----------------------------------------------------------------------------
# host golden model (mirrors device pipeline exactly, fp32, no hw noise)
def golden_loss(predictions, targets, k1=K1, k2=K2):
    mu_p, Cp = _unpack_row(predictions[0])
    mu_t, Ct = _unpack_row(targets[0])
    c1 = _power_iter_sym(Cp) * 1.05
    c2 = _power_iter_prod(Cp, Ct) * 1.10 / c1
    I = np.eye(D, dtype=np.float32)
    A1 = (Cp / c1 + EPS * I).astype(np.float32)
    q = A1.astype(BF16NP).astype(np.float32)
    r32 = A1 - q
    s = np.abs(r32).max(axis=1) * np.float32(1.0 / 127.0)
    s[s == 0] = 1.0
    qi8 = np.rint(r32 * (np.float32(1.0) / s)[:, None]).astype(np.int8)
    A1 = q + qi8.astype(np.float32) * s[:, None].astype(np.float32)
    Ctq = Ct.astype(BF16NP).astype(np.float32)

    def chain(A, sched):
        al, be = sched[0]
        T0 = (al * I + be * A).astype(np.float32)
        Y, Z = A @ T0, T0
        for alk, bek in sched[1:]:
            Pm = Z @ Y
            T = alk * I + bek * Pm
            Y, Z = Y @ T, T @ Z
        return Y, Z

    Y1, Z1 = chain(A1, make_schedule(EPS, 1.0 + EPS, k1))
    S = Y1 @ (1.5 * I - 0.5 * (Z1 @ Y1))
    V = (Ctq @ S) / c2
    A2 = (S @ V + EPS * I).astype(np.float32)
    Y2, Z2 = chain(A2, make_schedule(EPS, 1.0 + EPS, k2))
    U2 = Y2 @ Z2
    tr_corr = 1.5 * np.trace(Y2.astype(np.float64)) - 0.5 * float(
        np.sum(Y2.astype(np.float64) * U2.astype(np.float64)))
    tr_sqrtM = np.sqrt(c1 * c2) * tr_corr
    mu_term = float(np.mean((mu_p - mu_t) ** 2))
    return np.float32(mu_term + np.trace(Cp.astype(np.float64))
                      + np.trace(Ct.astype(np.float64)) + 2.0 * tr_sqrtM)


# ----------------------------------------------------------------------------
# dispatch: build program + jitted sharded executable once per process
def _get_runner():
    key = (K1, K2)
    if key in _RUN_CACHE:
        return _RUN_CACHE[key]

    import jax
    from jax.sharding import Mesh, PartitionSpec, NamedSharding
    from jax.experimental.shard_map import shard_map
    from concourse.bass2jax import (
        _bass_exec_p, partition_id_tensor, install_neuronx_cc_hook)

    nc = build_device_program(K1, K2)
    install_neuronx_cc_hook()

    partition_name = nc.partition_id_tensor.name if nc.partition_id_tensor else None
    in_names, out_names, out_avals, zero_shapes = [], [], [], []
    for alloc in nc.m.functions[0].allocations:
        if not isinstance(alloc, mybir.MemoryLocationSet):
            continue
        name = alloc.memorylocations[0].name
        if alloc.kind == "ExternalInput":
            if name != partition_name:
                in_names.append(name)
        elif alloc.kind == "ExternalOutput":
            shape = tuple(alloc.tensor_shape)
            dtype = mybir.dt.np(alloc.dtype)
            out_names.append(name)
            out_avals.append(jax.core.ShapedArray(shape, dtype))
            zero_shapes.append((shape, dtype))
    n_params = len(in_names)
    n_outs = len(out_avals)
    in_names_full = list(in_names) + out_names
    if partition_name is not None:
        in_names_full.append(partition_name)
    donate = tuple(range(n_params, n_params + n_outs))

    def _body(*args):
        operands = list(args)
        if partition_name is not None:
            operands.append(partition_id_tensor())
        outs = _bass_exec_p.bind(
            *operands,
            out_avals=tuple(out_avals),
            in_names=tuple(in_names_full),
            out_names=tuple(out_names),
            lowering_input_output_aliases=(),
            sim_require_finite=True,
            sim_require_nnan=True,
            nc=nc,
        )
        return tuple(outs)

    devices = jax.devices()[:NC]
    assert len(devices) == NC, f"need {NC} devices, got {len(jax.devices())}"
    mesh = Mesh(np.asarray(devices), ("core",))
    in_specs = (PartitionSpec("core"),) * (n_params + n_outs)
    out_specs = (PartitionSpec("core"),) * len(out_names)
    sharded = jax.jit(
        shard_map(_body, mesh=mesh, in_specs=in_specs, out_specs=out_specs,
                  check_rep=False),
        donate_argnums=donate,
        keep_unused=True,
    )
    shard_spec = NamedSharding(mesh, PartitionSpec("core"))

    # eps*I row slabs are input-independent: upload once, reuse every call.
    epsrow_np = np.zeros((D, D), np.float32)
    np.fill_diagonal(epsrow_np, EPS)
    epsrow_dev = jax.device_put(epsrow_np, shard_spec)

    runner = dict(
        sharded=sharded, in_names=in_names, out_names=out_names,
        zero_shapes=zero_shapes, epsrow=epsrow_dev, n_outs=n_outs,
        spec=shard_spec, device_put=jax.device_put,
    )
    _RUN_CACHE[key] = runner
    return runner


def kernel(predictions, targets):
    from concurrent.futures import ThreadPoolExecutor

    predictions = np.asarray(predictions)
    targets = np.asarray(targets)
    r = _get_runner()

    with ThreadPoolExecutor(2) as ex:
        fut_p = ex.submit(_unpack_row, predictions[0])
        fut_t = ex.submit(_unpack_row, targets[0])
        mu_t, Ct = fut_t.result()
        # prefetch Ct to the devices; the wire time overlaps the power
        # iterations and A1 prep below
        ct_dev = r["device_put"](Ct.astype(BF16NP), r["spec"])
        mu_p, Cp = fut_p.result()

    c1 = _power_iter_sym(Cp) * 1.05
    c2 = _power_iter_prod(Cp, Ct) * 1.10 / c1

    tr_cp = float(np.trace(Cp, dtype=np.float64))
    tr_ct = float(np.trace(Ct, dtype=np.float64))

    A1 = Cp * np.float32(1.0 / c1)
    didx = np.arange(D)
    A1[didx, didx] += np.float32(EPS)

    a1q = A1.astype(BF16NP)
    r32 = A1 - a1q.astype(np.float32)
    s = np.abs(r32).max(axis=1) * np.float32(1.0 / 127.0)
    s[s == 0] = 1.0
    a1res = np.rint(r32 * (np.float32(1.0) / s)[:, None]).astype(np.int8)
    a1sc = s.astype(np.float32).reshape(NC * SH, 1)
    invc2 = np.full((NC * P, 1), 1.0 / c2, np.float32)

    feed = {
        "a1q": a1q, "a1res": a1res, "a1sc": a1sc, "ctq": ct_dev,
        "epsrow": r["epsrow"], "invc2": invc2,
    }
    args = [feed[n] for n in r["in_names"]]
    zeros = [np.zeros((NC * s[0], *s[1:]), dt) for (s, dt) in r["zero_shapes"]]
    out = r["sharded"](*args, *zeros)
    parts = np.asarray(out[0]).reshape(NC, P, 8)

    syu = float(parts[:, :, 0:2].sum(dtype=np.float64))
    trY2 = float(parts[:, :, 2:4].sum(dtype=np.float64)) / EPS
    tr_corr = 1.5 * trY2 - 0.5 * syu
    tr_sqrtM = np.sqrt(c1 * c2) * tr_corr

    mu_term = float(np.mean((mu_p - mu_t) ** 2))
    loss = mu_term + tr_cp + tr_ct + 2.0 * tr_sqrtM
    return np.float32(loss)


# revision 15
# speedup vs baseline: 6.7341x; 1.1229x over previous
"""Trainium2 Bass kernel for nn_CustomLoss (2-Wasserstein-style Gaussian loss).

loss = mean((mu_p-mu_t)^2) + tr(Cp) + tr(Ct) + 2*tr(sqrtm(S2 @ Ct @ S2)),
       S2 = sqrtm(Cp),  d = 2048, packed inputs (4, 2100224), row 0 used.

Device algorithm: two scaled coupled Newton-Schulz sqrt chains in fp32r
(TensorEngine full rate), 8-way row-sharded across the NeuronCores with
AllGather (full operands for streaming) + AllToAll (column-slice delivery for
the stationary operand, avoiding core-dependent addressing in the SPMD
program). Scalar normalizers and the per-iteration scaling schedule are
host-side; the schedule is input-independent so one NEFF serves all inputs.

Dispatch path: the jitted PJRT executable is built once per process and
cached; per call only the input matrices move host->device, as fp16 row
slabs (golden-validated stable for the NS chains at eps=1e-4). Ct is
prefetched so its wire time hides under the host power iterations.
Column-slice (lhsT) operands are produced on device by PE transposes of
the row slabs (A1 and Ct are symmetric), and the eps*I row slab is
input-independent so it is uploaded once and kept device-resident.
"""
import numpy as np
import ml_dtypes

import concourse.bass as bass
import concourse.mybir as mybir
import concourse.tile as tile
from concourse.masks import make_identity

# Disable the walrus-embedded BIR simulator: ~4x faster NEFF compiles.
import concourse.bass_utils as _bu
if not getattr(_bu, "_nobirsim_patched", False):
    _orig_bvo = _bu.bir_verify_and_optimise

    def _bvo_fast(tmpdir, inp="bir.json", outp="file.neff", arch=None, *, dve_root=None):
        orig_run = _bu.run_command

        def patched_run(argv, **kw):
            argv = [a.replace("--enable-birsim=true", "--enable-birsim=false")
                    if isinstance(a, str) else a for a in argv]
            return orig_run(argv, **kw)

        _bu.run_command = patched_run
        try:
            return _orig_bvo(tmpdir, inp, outp, arch, dve_root=dve_root)
        finally:
            _bu.run_command = orig_run

    _bu.bir_verify_and_optimise = _bvo_fast
    _bu._nobirsim_patched = True

# ----------------------------------------------------------------------------
# config
D = 2048
NC = 8
SH = D // NC          # 256 rows per core
P = 128
KT = D // P           # 16 k-tiles
MB = SH // P          # 2 m-blocks per shard
NB = D // 512         # 4 n-blocks
CH = 2                # k-tiles per stream chunk
_TAG_BUFS = {"ostag": 2, "tstag": 2, "zstag": 1, "rstream": 2, "lhsT": 3}
EPS = 1e-4            # ridge (normalized units)
QCAP = 2.5            # max scaled eigenvalue (stability margin)
K1 = 9                # NS1 iterations (incl. cheap iter 1) + half-step
K2 = 10               # NS2 iterations (incl. cheap iter 1) + trace correction
F32 = mybir.dt.float32
F32R = mybir.dt.float32r
F16 = mybir.dt.float16
AF = mybir.ActivationFunctionType
ALU = mybir.AluOpType

_RUN_CACHE = {}


# ----------------------------------------------------------------------------
# host: schedule
def _f(q):
    return q * (3.0 - q) ** 2 / 4.0


def _balance_s(a, b, qcap):
    """s with f(s*a) = f(s*b), s*b <= qcap, via bisection."""
    s_hi = min(qcap, 2.9999) / b
    g = lambda s: _f(s * a) - _f(s * b)
    if g(s_hi) <= 0:
        return s_hi
    lo, hi = 1e-12, s_hi
    for _ in range(80):
        mid = 0.5 * (lo + hi)
        if g(mid) > 0:
            hi = mid
        else:
            lo = mid
    return 0.5 * (lo + hi)


def make_schedule(delta, b0, iters, qcap=QCAP):
    a, b = delta, b0
    out = []
    for _ in range(iters):
        s = 1.0 if a > 0.99 * b else _balance_s(a, b, qcap)
        mu = np.sqrt(s)
        out.append((1.5 * mu, -0.5 * mu ** 3))   # (alpha, beta): T = a*I + b*P
        qa, qb = s * a, s * b
        vals = [_f(qa), _f(qb)]
        b = 1.0 if qa <= 1.0 <= qb else max(vals)
        a = min(vals)
    return out


# ----------------------------------------------------------------------------
# host: input prep
_TRI_OFF = np.concatenate([[0], np.cumsum(D - np.arange(D))]).astype(np.int64)


def _unpack_row(v):
    """Packed row -> (mu[D] f64, C[D,D] f32 symmetric). Row-slice loop +
    transpose-add: ~5x faster than a triu fancy-index scatter."""
    mu = np.asarray(v[:D], dtype=np.float64)
    tri = np.asarray(v[D:], dtype=np.float32)
    U = np.zeros((D, D), np.float32)
    off = _TRI_OFF
    for i in range(D):
        U[i, i:] = tri[off[i]:off[i + 1]]
    d = np.diagonal(U).copy()
    C = U + U.T
    np.fill_diagonal(C, d)
    return mu, C


def _power_iter_sym(C, iters=25):
    rng = np.random.default_rng(12345)
    x = rng.standard_normal(D).astype(np.float32)
    lam = 1.0
    for _ in range(iters):
        y = C @ x
        lam = np.linalg.norm(y)
        x = y / lam
    return float(lam)


def _power_iter_prod(Cp, Ct, iters=20):
    rng = np.random.default_rng(54321)
    x = rng.standard_normal(D).astype(np.float32)
    lam = 1.0
    for _ in range(iters):
        y = Cp @ (Ct @ x)
        lam = np.linalg.norm(y)
        x = y / lam
    return float(lam)


# ----------------------------------------------------------------------------
# walrus workaround: this build allows only ONE sync-wait per instruction
class PatchedTileContext(tile.TileContext):
    def _drain_and_barrier(self, tick_clock, wait_clock):
        from concourse.vector_clock import ScopedClock

        probe = self.nc.sync.nop(nofuse=True)
        wait_clock.add_sem_waits(
            probe.ins, ScopedClock({None: tick_clock.global_clock})
        )
        si = probe.ins.sync_info
        waits = list(si.on_wait) if si is not None else []
        if len(waits) > 1:
            si.on_wait = [waits[0]]
            for w in waits[1:]:
                n2 = self.nc.sync.nop(nofuse=True)
                si2 = n2.ins.sync_info
                if si2 is None:
                    n2.ins.sync_info = mybir.SyncInfo(on_wait=[w], on_update=[])
                else:
                    si2.on_wait = [w]
        self.nc.sync.drain()
        self.nc.all_engine_barrier()
        assert self.sems is not None
        popped = self.nc._tile_sem_poison_stack.pop()
        assert popped is self._sem_poison
        self.nc.clear_and_free_semaphores(list(self.sems.allocated().values()))
        self.nc.all_engine_barrier()


def legalize_single_wait(nc):
    uid = 0
    for fn in nc.m.functions:
        for blk in fn.blocks:
            il = blk.instructions
            if not any(
                i.sync_info is not None and len(i.sync_info.on_wait) > 1 for i in il
            ):
                continue
            new = []
            for ins in il:
                si = ins.sync_info
                waits = list(si.on_wait) if si is not None else []
                if len(waits) > 1:
                    si.on_wait = [waits[-1]]
                    for w in waits[:-1]:
                        nop = mybir.InstNoOp(
                            name=f"legalize-wait-{uid}",
                            engine=ins.engine,
                            sync_info=mybir.SyncInfo(on_wait=[w], on_update=[]),
                        )
                        uid += 1
                        new.append(nop)
                new.append(ins)
            blk.instructions = new


# ----------------------------------------------------------------------------
# device program builder
class _B:
    """Builder state."""

    def __init__(self, nc, tc, dram, sb, psum):
        self.nc, self.tc = nc, tc
        self.dram, self.sb, self.psum = dram, sb, psum
        self.uid = 0
        self.ident = None    # [P, P] identity f32
        self.epsrow = None   # [P, MB, D] eps*I row slab (per-core input)

    def u(self, s):
        self.uid += 1
        return f"{s}_{self.uid}"


def _stream_view(full_ap):
    """[D, D] dram AP -> [P, NCH, CH, D] chunked k-tile stream view."""
    return full_ap.rearrange("(ch kb p) n -> p ch kb n", p=P, kb=CH)


def _lhsT_view(a2a_ap):
    """[D, SH] dram AP (A2A out, flat) -> [P, KT, SH]."""
    return a2a_ap.rearrange("(k p) m -> p k m", p=P)


def _mm_shard(b: _B, lhsT_sb, rhs_chunks, scale, eps_coef, tag="ostag"):
    """out_stag[P, MB, D] = (lhsT^T @ rhs) * scale (+ eps_coef * epsrow).

    lhsT_sb: [P, KT, SH] f32 sbuf; rhs_chunks: [P, NCHUNK, CH, D] dram view.
    scale: float or AP. eps_coef: None or float g (adds g * epsrow).
    """
    nc = b.nc
    stag = b.sb.tile([P, MB, D], F32R, tag=tag, name=b.u(tag), bufs=_TAG_BUFS[tag])
    ps = [
        b.psum.tile([P, 512], F32, tag="mmps", name=b.u("ps"))
        for _ in range(MB * NB)
    ]
    for ch in range(KT // CH):
        rt = b.sb.tile([P, CH, D], F32R, tag="rstream", name=b.u("rt"), bufs=_TAG_BUFS["rstream"])
        nc.sync.dma_start(out=rt[:], in_=rhs_chunks[:, ch])
        for kk in range(CH):
            k = ch * CH + kk
            for m in range(MB):
                for n in range(NB):
                    nc.tensor.matmul(
                        ps[m * NB + n][:],
                        lhsT_sb[:, k, m * P:(m + 1) * P],
                        rt[:, kk, n * 512:(n + 1) * 512],
                        start=(k == 0),
                        stop=(k == KT - 1),
                    )
    for m in range(MB):
        for n in range(NB):
            if eps_coef is not None:
                # add (eps_coef/scale) * epsrow into psum pre-eviction so the
                # scaled eviction yields  scale*psum + eps_coef*epsrow
                nc.vector.scalar_tensor_tensor(
                    ps[m * NB + n][:],
                    b.epsrow[:, m, n * 512:(n + 1) * 512],
                    float(eps_coef) / _scale_const(scale),
                    ps[m * NB + n][:],
                    ALU.mult,
                    ALU.add,
                )
            nc.scalar.activation(
                stag[:, m, n * 512:(n + 1) * 512],
                ps[m * NB + n][:],
                AF.Copy,
                scale=scale,
            )
    return stag


def _scale_const(scale):
    assert isinstance(scale, (int, float)), "eps_coef requires constant scale"
    return float(scale)


def _transpose_shard(b: _B, stag):
    """[P, MB, D] staging (rows shard of X) -> [P, KT, SH] = X^T[:, shard cols]."""
    nc = b.nc
    tt = b.sb.tile([P, KT, SH], F32R, tag="lhsT", name=b.u("tt"), bufs=_TAG_BUFS["lhsT"])
    for k in range(KT):
        for m in range(MB):
            tp = b.psum.tile([P, 512], F32R, tag="mmps", name=b.u("tps"))
            nc.tensor.transpose(
                tp[:, 0:P], stag[:, m, k * P:(k + 1) * P], b.ident[:]
            )
            nc.scalar.copy(tt[:, k, m * P:(m + 1) * P], tp[:, 0:P])
    return tt


def _load_lhsT(b: _B, dram_flat_ap):
    """DMA [D, SH] dram -> [P, KT, SH] sbuf."""
    t = b.sb.tile([P, KT, SH], F32R, tag="lhsT", name=b.u("lh"), bufs=_TAG_BUFS["lhsT"])
    b.nc.sync.dma_start(out=t[:], in_=_lhsT_view(dram_flat_ap))
    return t


def _bounce_and_gather(b: _B, stag, want_a2a, name):
    """Write staging to DRAM, AllGather full (+ optionally AllToAll col-slice).

    Returns (full_dram_ap [D, D], a2a_out_ap [D, SH] or None).
    """
    nc = b.nc
    bounce = b.dram.tile([SH, D], F32R, name=b.u(f"bn_{name}"), tag="d_bn", bufs=4)
    nc.gpsimd.dma_start(
        out=bounce[:].rearrange("(m p) n -> p m n", p=P), in_=stag[:]
    )
    full = b.dram.tile([D, D], F32R, name=b.u(f"fl_{name}"), addr_space="Shared", tag="d_fl", bufs=4)
    nc.gpsimd.collective_compute(
        "AllGather",
        ALU.bypass,
        replica_groups=[list(range(NC))],
        ins=[bounce[:]],
        outs=[full[:]],
    )
    a2a_out = None
    if want_a2a:
        a2a_in = b.dram.tile([NC, SH, SH], F32R, name=b.u(f"ai_{name}"), tag="d_ai", bufs=4)
        for j in range(NC):
            nc.gpsimd.dma_start(
                out=a2a_in[j].rearrange("(m p) n -> p m n", p=P),
                in_=stag[:, :, j * SH:(j + 1) * SH],
            )
        a2a_out = b.dram.tile([NC * SH, SH], F32R, name=b.u(f"ao_{name}"), tag="d_ao", bufs=4)
        nc.gpsimd.collective_compute(
            "AllToAll",
            ALU.bypass,
            replica_groups=[list(range(NC))],
            ins=[a2a_in[:]],
            outs=[a2a_out[:]],
        )
    return full[:], (a2a_out[:] if a2a_out is not None else None)


def _ns_chain(b: _B, a_col_lhsT_sb, a_row_stag, sched, name):
    """Run a scaled NS chain. Inputs:
      a_col_lhsT_sb: [P, KT, SH] sbuf = A[:, shard cols]  (lhsT of A)
      a_row_stag:    [P, MB, D] sbuf = A[shard rows, :]   (row slab of A)
    Returns dict with Yfull, Zfull (dram APs), Y_a2a, Z_a2a, Y_stag (sbuf).
    """
    nc = b.nc
    al0, be0 = sched[0]
    # iter 1: T0 = al0*I + be0*A (sharded, elementwise); Z1 = T0; Y1 = A @ T0
    t0f = b.sb.tile([P, MB, D], F32, tag="f32tmp", name=b.u("t0f"), bufs=1)
    t0 = b.sb.tile([P, MB, D], F32R, tag="ostag", name=b.u("t0"), bufs=_TAG_BUFS["ostag"])
    for m in range(MB):
        nc.scalar.mul(t0f[:, m, :], a_row_stag[:, m, :].bitcast(F32), float(be0))
        nc.vector.scalar_tensor_tensor(
            t0f[:, m, :], b.epsrow[:, m, :], float(al0 / EPS),
            t0f[:, m, :], ALU.mult, ALU.add,
        )
        nc.scalar.copy(t0[:, m, :], t0f[:, m, :])
    t0_full, t0_a2a = _bounce_and_gather(b, t0, True, f"{name}t0")
    y_stag = _mm_shard(b, a_col_lhsT_sb, _stream_view(t0_full), 1.0, None)
    y_full, y_a2a = _bounce_and_gather(b, y_stag, True, f"{name}y1")
    st = dict(Yfull=y_full, Y_a2a=y_a2a, Zfull=t0_full, Z_a2a=t0_a2a, Y_stag=y_stag)

    for it in range(1, len(sched)):
        al, be = sched[it]
        lh_z = _get_lhsT(b, st, "Z")
        lh_y = _get_lhsT(b, st, "Y")
        # P = Z @ Y ; T = al*I + be*P  (keep T staging for local transpose)
        t_stag = _mm_shard(b, lh_z, _get_stream(b, st, "Y"), float(be), al / EPS,
                           tag="tstag")
        t_full, _ = _bounce_and_gather(b, t_stag, False, f"{name}t{it}")
        # Z' = T @ Z : lhsT = T^T[:, shard] = transpose of own T staging
        lh_tt = _transpose_shard(b, t_stag)
        z_stag = _mm_shard(b, lh_tt, _get_stream(b, st, "Z"), 1.0, None,
                           tag="zstag")
        # Y' = Y @ T
        y_stag = _mm_shard(b, lh_y, _stream_view(t_full), 1.0, None)
        # batched gather of (Y', Z')
        bounce = b.dram.tile([2 * SH, D], F32R, name=b.u("bnyz"), tag="d_bnyz", bufs=4)
        nc.gpsimd.dma_start(
            out=bounce[:].rearrange("(t m p) n -> t p m n", t=2, p=P)[0],
            in_=y_stag[:])
        nc.gpsimd.dma_start(
            out=bounce[:].rearrange("(t m p) n -> t p m n", t=2, p=P)[1],
            in_=z_stag[:])
        full = b.dram.tile([NC * 2 * SH, D], F32R, name=b.u("flyz"),
                           addr_space="Shared", tag="d_flyz", bufs=4)
        nc.gpsimd.collective_compute(
            "AllGather", ALU.bypass, replica_groups=[list(range(NC))],
            ins=[bounce[:]], outs=[full[:]],
        )
        a2a_in = b.dram.tile([NC, 2, SH, SH], F32R, name=b.u("aiyz"), tag="d_aiyz", bufs=4)
        for j in range(NC):
            nc.gpsimd.dma_start(
                out=a2a_in[j, 0].rearrange("(m p) n -> p m n", p=P),
                in_=y_stag[:, :, j * SH:(j + 1) * SH])
            nc.gpsimd.dma_start(
                out=a2a_in[j, 1].rearrange("(m p) n -> p m n", p=P),
                in_=z_stag[:, :, j * SH:(j + 1) * SH])
        a2a_out = b.dram.tile([NC, 2, SH, SH], F32R, name=b.u("aoyz"), tag="d_aoyz", bufs=4)
        nc.gpsimd.collective_compute(
            "AllToAll", ALU.bypass, replica_groups=[list(range(NC))],
            ins=[a2a_in[:]], outs=[a2a_out[:]],
        )
        # views: full rows = (c, t, m p); Y = t 0, Z = t 1
        fv = full[:].rearrange("(c t kb p) n -> t p c kb n", t=2, kb=CH, p=P)
        av = a2a_out[:].rearrange("s t (kb p) m -> t p s kb m", kb=CH, p=P)
        st = dict(
            Yfull=fv[0], Zfull=fv[1],           # [P, NC, CH, D] chunk views
            Y_a2a=av[0], Z_a2a=av[1],           # [P, s, kb, SH] 4d lhsT views
            Y_stag=y_stag, Z_stag=z_stag,
            chunked=True,
        )
    return st


def _load_lhsT4(b: _B, view4):
    """DMA [P, s, kb, SH] 4d view -> [P, KT, SH] sbuf (k = s*CH + kb)."""
    t = b.sb.tile([P, KT, SH], F32R, tag="lhsT", name=b.u("lh4"), bufs=_TAG_BUFS["lhsT"])
    for s in range(NC):
        b.nc.sync.dma_start(
            out=t[:, s * CH:(s + 1) * CH, :], in_=view4[:, s]
        )
    return t


def _get_lhsT(b, st, key):
    v = st[f"{key}_a2a"]
    if st.get("chunked"):
        return _load_lhsT4(b, v)
    return _load_lhsT(b, v)


def _get_stream(b, st, key):
    v = st[f"{key}full"]
    if st.get("chunked"):
        return v
    return _stream_view(v)


def _load_qrow(b: _B, base_d):
    """DMA an fp16 row slab and convert to an F32R [P, MB, D] staging tile."""
    nc = b.nc
    q = b.sb.tile([P, MB, D], F16, tag="qin", name=b.u("qin"), bufs=1)
    nc.sync.dma_start(out=q[:], in_=base_d[:].rearrange("(m p) n -> p m n", p=P))
    stag = b.sb.tile([P, MB, D], F32R, tag="ostag", name=b.u("qrow"),
                     bufs=_TAG_BUFS["ostag"])
    for m in range(MB):
        nc.scalar.copy(stag[:, m, :], q[:, m, :])
    return stag


def build_device_program(k1, k2):
    sched1 = make_schedule(EPS, 1.0 + EPS, k1)
    sched2 = make_schedule(EPS, 1.0 + EPS, k2)

    nc = bass.Bass(num_devices=NC)
    with PatchedTileContext(nc) as tc:
        with tc.tile_pool(name="dram", bufs=1, space="DRAM") as dram, \
             tc.tile_pool(name="sb", bufs=1) as sb_const, \
             tc.tile_pool(name="sbw", bufs=3) as sbw, \
             tc.tile_pool(name="psum", bufs=8, space="PSUM") as psum:

            b = _B(nc, tc, dram, sbw, psum)

            # --- inputs (a1 double-bf16, ct single bf16, epsrow resident f32)
            a1h = dram.tile([SH, D], F16, kind="ExternalInput", name="a1h", uniquify=False)
            cth = dram.tile([SH, D], F16, kind="ExternalInput", name="cth", uniquify=False)
            epsrow_d = dram.tile([SH, D], F32, kind="ExternalInput", name="epsrow", uniquify=False)
            invc2_d = dram.tile([P, 1], F32, kind="ExternalInput", name="invc2", uniquify=False)
            partials_d = dram.tile([P, 8], F32, kind="ExternalOutput", name="partials", uniquify=False)

            # --- constants resident in SBUF
            ident_f = sb_const.tile([P, P], F32, name="ident_f", uniquify=False)
            make_identity(nc, ident_f[:])
            ident = sb_const.tile([P, P], F32R, name="ident", uniquify=False)
            nc.scalar.copy(ident[:], ident_f[:])
            b.ident = ident
            epsrow = sb_const.tile([P, MB, D], F32, name="epsrow_sb", uniquify=False)
            nc.sync.dma_start(out=epsrow[:], in_=epsrow_d[:].rearrange("(m p) n -> p m n", p=P))
            b.epsrow = epsrow
            invc2 = sb_const.tile([P, 1], F32, name="invc2_sb", uniquify=False)
            nc.sync.dma_start(out=invc2[:], in_=invc2_d[:])
            part = sb_const.tile([P, 8], F32, name="part_sb", uniquify=False)
            b.part = part

            # --- NS1 on A1 (double-bf16 upload; row slab -> local transpose
            # for the column-slice lhsT since A1 is symmetric)
            a1r_stag = _load_qrow(b, a1h)
            a1c_sb = _transpose_shard(b, a1r_stag)
            st1 = _ns_chain(b, a1c_sb, a1r_stag, sched1, "n1")

            # --- NS1 half-step: S = Y*(1.5 I - 0.5 Z Y)
            lh_z = _get_lhsT(b, st1, "Z")
            lh_y = _get_lhsT(b, st1, "Y")
            tp_stag = _mm_shard(b, lh_z, _get_stream(b, st1, "Y"), -0.5, 1.5 / EPS,
                                tag="tstag")
            tp_full, _ = _bounce_and_gather(b, tp_stag, False, "half")
            s_stag = _mm_shard(b, lh_y, _stream_view(tp_full), 1.0, None)
            s_full, s_a2a = _bounce_and_gather(b, s_stag, True, "sfin")

            # --- middle: V = (Ct @ S)/c2 ; A2 = S @ V + eps I
            ct_stag = _load_qrow(b, cth)
            ct_sb = _transpose_shard(b, ct_stag)
            v_stag = _mm_shard(b, ct_sb, _stream_view(s_full), invc2[:, 0:1],
                               None, tag="tstag")
            v_full, _ = _bounce_and_gather(b, v_stag, False, "vmid")
            lh_s = _load_lhsT(b, s_a2a)
            a2_stag = _mm_shard(b, lh_s, _stream_view(v_full), 1.0, 1.0)
            # A2: only A2A needed (lhsT for NS2 iter1); row slab is local staging
            a2a_in = b.dram.tile([NC, SH, SH], F32R, name=b.u("ai_a2"), tag="d_ai", bufs=4)
            for j in range(NC):
                nc.gpsimd.dma_start(
                    out=a2a_in[j].rearrange("(m p) n -> p m n", p=P),
                    in_=a2_stag[:, :, j * SH:(j + 1) * SH])
            a2_a2a = b.dram.tile([NC * SH, SH], F32R, name=b.u("ao_a2"), tag="d_ao", bufs=4)
            nc.gpsimd.collective_compute(
                "AllToAll", ALU.bypass, replica_groups=[list(range(NC))],
                ins=[a2a_in[:]], outs=[a2_a2a[:]],
            )
            a2c_sb = _load_lhsT(b, a2_a2a[:])

            # --- NS2
            st2 = _ns_chain(b, a2c_sb, a2_stag, sched2, "n2")

            # --- trace stage: U2 = Y2 @ Z2 (staging only)
            lh_y2 = _get_lhsT(b, st2, "Y")
            u2_stag = _mm_shard(b, lh_y2, _get_stream(b, st2, "Z"), 1.0, None,
                                tag="tstag")
            y2_stag = st2["Y_stag"]
            part = b.part
            nc.gpsimd.memset(part[:], 0.0)
            tmp = b.sb.tile([P, MB, D], F32, tag="f32tmp", name=b.u("tmp"), bufs=1)
            for m in range(MB):
                nc.vector.tensor_mul(
                    tmp[:, m, :], y2_stag[:, m, :].bitcast(F32),
                    u2_stag[:, m, :].bitcast(F32))
                nc.vector.tensor_reduce(
                    part[:, m:m + 1], tmp[:, m, :], mybir.AxisListType.X, ALU.add)
                nc.vector.tensor_mul(
                    tmp[:, m, :], y2_stag[:, m, :].bitcast(F32), epsrow[:, m, :])
                nc.vector.tensor_reduce(
                    part[:, 2 + m:3 + m], tmp[:, m, :], mybir.AxisListType.X, ALU.add)
            nc.sync.dma_start(out=partials_d[:], in_=part[:])

    legalize_single_wait(nc)
    return nc


# ----------------------------------------------------------------------------
# host golden model (mirrors device pipeline exactly, fp32, no hw noise)
def golden_loss(predictions, targets, k1=K1, k2=K2):
    mu_p, Cp = _unpack_row(predictions[0])
    mu_t, Ct = _unpack_row(targets[0])
    c1 = _power_iter_sym(Cp) * 1.05
    c2 = _power_iter_prod(Cp, Ct) * 1.10 / c1
    I = np.eye(D, dtype=np.float32)
    A1 = (Cp / c1 + EPS * I).astype(np.float32)
    A1 = A1.astype(np.float16).astype(np.float32)
    Ctq = Ct.astype(np.float16).astype(np.float32)

    def chain(A, sched):
        al, be = sched[0]
        T0 = (al * I + be * A).astype(np.float32)
        Y, Z = A @ T0, T0
        for alk, bek in sched[1:]:
            Pm = Z @ Y
            T = alk * I + bek * Pm
            Y, Z = Y @ T, T @ Z
        return Y, Z

    Y1, Z1 = chain(A1, make_schedule(EPS, 1.0 + EPS, k1))
    S = Y1 @ (1.5 * I - 0.5 * (Z1 @ Y1))
    V = (Ctq @ S) / c2
    A2 = (S @ V + EPS * I).astype(np.float32)
    Y2, Z2 = chain(A2, make_schedule(EPS, 1.0 + EPS, k2))
    U2 = Y2 @ Z2
    tr_corr = 1.5 * np.trace(Y2.astype(np.float64)) - 0.5 * float(
        np.sum(Y2.astype(np.float64) * U2.astype(np.float64)))
    tr_sqrtM = np.sqrt(c1 * c2) * tr_corr
    mu_term = float(np.mean((mu_p - mu_t) ** 2))
    return np.float32(mu_term + np.trace(Cp.astype(np.float64))
                      + np.trace(Ct.astype(np.float64)) + 2.0 * tr_sqrtM)


# ----------------------------------------------------------------------------
# dispatch: build program + jitted sharded executable once per process
def _get_runner():
    key = (K1, K2)
    if key in _RUN_CACHE:
        return _RUN_CACHE[key]

    import jax
    from jax.sharding import Mesh, PartitionSpec, NamedSharding
    from jax.experimental.shard_map import shard_map
    from concourse.bass2jax import (
        _bass_exec_p, partition_id_tensor, install_neuronx_cc_hook)

    nc = build_device_program(K1, K2)
    install_neuronx_cc_hook()

    partition_name = nc.partition_id_tensor.name if nc.partition_id_tensor else None
    in_names, out_names, out_avals, zero_shapes = [], [], [], []
    for alloc in nc.m.functions[0].allocations:
        if not isinstance(alloc, mybir.MemoryLocationSet):
            continue
        name = alloc.memorylocations[0].name
        if alloc.kind == "ExternalInput":
            if name != partition_name:
                in_names.append(name)
        elif alloc.kind == "ExternalOutput":
            shape = tuple(alloc.tensor_shape)
            dtype = mybir.dt.np(alloc.dtype)
            out_names.append(name)
            out_avals.append(jax.core.ShapedArray(shape, dtype))
            zero_shapes.append((shape, dtype))
    n_params = len(in_names)
    n_outs = len(out_avals)
    in_names_full = list(in_names) + out_names
    if partition_name is not None:
        in_names_full.append(partition_name)
    donate = tuple(range(n_params, n_params + n_outs))

    def _body(*args):
        operands = list(args)
        if partition_name is not None:
            operands.append(partition_id_tensor())
        outs = _bass_exec_p.bind(
            *operands,
            out_avals=tuple(out_avals),
            in_names=tuple(in_names_full),
            out_names=tuple(out_names),
            lowering_input_output_aliases=(),
            sim_require_finite=True,
            sim_require_nnan=True,
            nc=nc,
        )
        return tuple(outs)

    devices = jax.devices()[:NC]
    assert len(devices) == NC, f"need {NC} devices, got {len(jax.devices())}"
    mesh = Mesh(np.asarray(devices), ("core",))
    in_specs = (PartitionSpec("core"),) * (n_params + n_outs)
    out_specs = (PartitionSpec("core"),) * len(out_names)
    sharded = jax.jit(
        shard_map(_body, mesh=mesh, in_specs=in_specs, out_specs=out_specs,
                  check_rep=False),
        donate_argnums=donate,
        keep_unused=True,
    )
    shard_spec = NamedSharding(mesh, PartitionSpec("core"))

    # eps*I row slabs are input-independent: upload once, reuse every call.
    epsrow_np = np.zeros((D, D), np.float32)
    np.fill_diagonal(epsrow_np, EPS)
    epsrow_dev = jax.device_put(epsrow_np, shard_spec)

    runner = dict(
        sharded=sharded, in_names=in_names, out_names=out_names,
        zero_shapes=zero_shapes, epsrow=epsrow_dev, n_outs=n_outs,
        spec=shard_spec, device_put=jax.device_put,
    )
    _RUN_CACHE[key] = runner
    return runner


def kernel(predictions, targets):
    from concurrent.futures import ThreadPoolExecutor

    predictions = np.asarray(predictions)
    targets = np.asarray(targets)
    r = _get_runner()

    with ThreadPoolExecutor(2) as ex:
        fut_p = ex.submit(_unpack_row, predictions[0])
        fut_t = ex.submit(_unpack_row, targets[0])
        mu_t, Ct = fut_t.result()
        # prefetch Ct to the devices; the wire time overlaps the power
        # iterations and A1 prep below
        ct_dev = r["device_put"](Ct.astype(np.float16), r["spec"])
        mu_p, Cp = fut_p.result()

    c1 = _power_iter_sym(Cp) * 1.05
    c2 = _power_iter_prod(Cp, Ct) * 1.10 / c1

    tr_cp = float(np.trace(Cp, dtype=np.float64))
    tr_ct = float(np.trace(Ct, dtype=np.float64))

    A1 = Cp * np.float32(1.0 / c1)
    didx = np.arange(D)
    A1[didx, didx] += np.float32(EPS)

    a1h = A1.astype(np.float16)
    invc2 = np.full((NC * P, 1), 1.0 / c2, np.float32)

    feed = {
        "a1h": a1h, "cth": ct_dev,
        "epsrow": r["epsrow"], "invc2": invc2,
    }
    args = [feed[n] for n in r["in_names"]]
    zeros = [np.zeros((NC * s[0], *s[1:]), dt) for (s, dt) in r["zero_shapes"]]
    out = r["sharded"](*args, *zeros)
    parts = np.asarray(out[0]).reshape(NC, P, 8)

    syu = float(parts[:, :, 0:2].sum(dtype=np.float64))
    trY2 = float(parts[:, :, 2:4].sum(dtype=np.float64)) / EPS
    tr_corr = 1.5 * trY2 - 0.5 * syu
    tr_sqrtM = np.sqrt(c1 * c2) * tr_corr

    mu_term = float(np.mean((mu_p - mu_t) ** 2))
    loss = mu_term + tr_cp + tr_ct + 2.0 * tr_sqrtM
    return np.float32(loss)
